# revision 12
# baseline (speedup 1.0000x reference)
"""BlockGlobalAttentionProduct Trainium2 kernel (v2).

Sharding: 24 (n,h) pairs across 8 cores, 3 per core. Per (n,h):
  - ONE dma_gather table per head: 512B rows [V1 bf16 130B | K_hi | K_lo |
    K_hi fp8 64B each | pad] (a 512B descriptor costs the same as 256B),
    8192 slots (local 4096 + global 4096) in 2 calls; V1 (with baked ones
    column) is used directly as the PV rhs
  - K^T built by one PE transpose per tile at bf16 granularity (2 fp8
    d-values ride one 16-bit unit), yielding a [96, 2-plane] layout
    (K_hi, K_lo, K_hi) + 3 bias/mask partitions in one copy
  - scores run as fp8 DoubleRow matmuls (0.5 cyc/col) with hi/lo error
    compensation: q ships as (q_hi, q_hi, q_lo) plane pairs so the matmul
    computes q_hi*K_hi + q_hi*K_lo + q_lo*K_hi ~ exact q.K; the local halo
    mask and a -2.5 exp bias fold in as extra partition pairs
  - exp(0.125 s) computed on THREE engines: ScalarE activation, DVE pow, and
    GPSIMD pow (base = e^0.125 broadcast), outputs fp8 probs
  - PV transposed: out[q, 65] += E[keys, q].T @ V1[keys, 65] per (tile,
    q-tile) incidence; gtok contribution initializes each q-tile with the
    gv1 parity trick; ctx copies (PSUM->SBUF bf16) split across DVE/GPSIMD
  - host does the final divide-by-denominator
"""

import sys

sys.path.insert(0, "/opt/trn_rl_repo")

import numpy as np
import ml_dtypes

import concourse.bacc as bacc
import concourse.mybir as mybir
from concourse import bass, tile, bass_utils, library_config

N, H, T, D = 2, 12, 4096, 64
NH = N * H
NCORES = 8
PER_CORE = NH // NCORES   # 3
NT = 32                   # 128-key tiles per table
QW = 128 + T + 128        # q halo cols [-128, T+128)

BF16 = mybir.dt.bfloat16
F32 = mybir.dt.float32
FP8 = mybir.dt.float8e4
I16 = mybir.dt.int16
EXP = mybir.ActivationFunctionType.Exp
DR = mybir.MatmulPerfMode.DoubleRow
POW = mybir.AluOpType.pow

# engine assignment for exp packs / ctx copies: "S" ScalarE, "D" DVE, "P" Pool
L_ENG = ["S", "D", "P", "S", "S", "D", "P", "S"]            # 8 packs of 1024
G_ENG = ["S", "S", "D", "S", "S", "P", "S", "S",
         "D", "S", "S", "P", "S", "S", "D", "P"]            # 16 packs of 768
T_ENG = ["S", "S", "S", "S"]                                # 4 packs of 512
C_ENG = ["D", "P", "D", "P", "D", "P", "D", "D"]            # 8 ctx copies of 260


def build_program():
    nc = bacc.Bacc("TRN2", target_bir_lowering=False, debug=False,
                   num_devices=NCORES)

    qTh_d = nc.dram_tensor("qTh", [PER_CORE, 99, 2, QW], FP8, kind="ExternalInput")
    kv_d = nc.dram_tensor("kv", [PER_CORE, T, 256], BF16, kind="ExternalInput")
    kmask_d = nc.dram_tensor("kmask", [3, T], BF16, kind="ExternalInput")
    gkTp_d = nc.dram_tensor("gkTp", [PER_CORE, 97, 2, 64], FP8, kind="ExternalInput")
    gv1_d = nc.dram_tensor("gv1", [PER_CORE, 128, 2, 65], BF16, kind="ExternalInput")
    idx_d = nc.dram_tensor("idx", [128, PER_CORE * 512], I16, kind="ExternalInput")
    ident_d = nc.dram_tensor("ident", [128, 128], BF16, kind="ExternalInput")
    base_d = nc.dram_tensor("base", [128, 1], F32, kind="ExternalInput")
    out_d = nc.dram_tensor("ctxT", [PER_CORE, 128, 32, 65], BF16,
                           kind="ExternalOutput")

    with tile.TileContext(nc) as tc:
        with (
            tc.tile_pool(name="const", bufs=1) as constp,
            tc.tile_pool(name="land", bufs=2) as land,
            tc.tile_pool(name="kt", bufs=2) as ktp,
            tc.tile_pool(name="expa", bufs=2) as expa,
            tc.tile_pool(name="expb", bufs=1) as expb,
            tc.tile_pool(name="outp", bufs=2) as outp,
            tc.tile_pool(name="psL", bufs=1, space="PSUM") as psL,
            tc.tile_pool(name="psG", bufs=1, space="PSUM") as psG,
            tc.tile_pool(name="psT", bufs=1, space="PSUM") as psT,
            tc.tile_pool(name="psPV", bufs=2, space="PSUM") as psPV,
            tc.tile_pool(name="aux", bufs=1, space="PSUM") as auxp,
        ):
            ident = constp.tile([128, 128], BF16, tag="ident")
            base = constp.tile([128, 1], F32, tag="base")
            idx_sb = constp.tile([128, PER_CORE * 512], I16, tag="idx")
            nc.sync.dma_start(idx_sb[:, 0:768], idx_d[:, 0:768])
            nc.sync.dma_start(idx_sb[:, 768:], idx_d[:, 768:])
            nc.sync.dma_start(ident[:], ident_d[:])
            nc.sync.dma_start(base[:], base_d[:])
            lib_i = nc.gpsimd.load_library(library_config.mlp)

            first_gather = [None]
            last_gather = [None]

            def emit_loads(i, part=False):
                """DMA loads + 2 gather calls for head i."""
                from concourse.tile_rust import add_dep_helper
                q34 = land.tile([99, 2, QW], FP8, tag="q")
                kv = land.tile([128, 64, 256], BF16, tag="kv")
                gkTp = land.tile([97, 2, 64], FP8, tag="gkTp")
                gv1 = land.tile([128, 2, 65], BF16, tag="gv1")
                d1 = nc.sync.dma_start(q34[:], qTh_d[i])
                d2 = nc.sync.dma_start(gkTp[:], gkTp_d[i])
                d3 = nc.sync.dma_start(gv1[:], gv1_d[i])
                if last_gather[0] is not None:
                    for d in (d1, d2, d3):
                        add_dep_helper(d.ins, last_gather[0].ins,
                                       reason="prev gathers before next loads")
                gs = []
                for t in range(2):   # L then G table
                    g = nc.gpsimd.dma_gather(
                        kv[:, 32 * t:32 * t + 32, :], kv_d[i],
                        idx_sb[:, 512 * i + 256 * t:512 * i + 256 * t + 256],
                        4096, 4096, 256, single_packet=False)
                    gs.append(g)
                if first_gather[0] is None:
                    add_dep_helper(lib_i.ins, gs[0].ins, reason="lib first")
                    first_gather[0] = gs[0]
                last_gather[0] = gs[-1]
                return dict(q34=q34, kv=kv, gkTp=gkTp, gv1=gv1)

            class NHState:
                def __init__(self, i, loads):
                    self.i = i
                    self.q34 = loads["q34"]
                    self.kv = loads["kv"]
                    self.gkTp = loads["gkTp"]
                    self.gv1 = loads["gv1"]
                    # K^T tables, bf16-unit layout [34, T]; fp8 view is
                    # [34, 2, T]-plane DoubleRow weights
                    self.klT = ktp.tile([99, T], BF16, tag="klT")
                    self.kgT = ktp.tile([97, T], BF16, tag="kgT")
                    nc.sync.dma_start(self.klT[96:99, :], kmask_d[:])
                    nc.sync.dma_start(self.kgT[96:97, :], kmask_d[0:1, :])
                    self.ELa = expa.tile([128, 16, 256], BF16, tag="ELa")
                    self.ELb = expb.tile([128, 16, 256], BF16, tag="ELb")
                    self.EGa = expa.tile([128, 16, 384], BF16, tag="EGa")
                    self.EGb = expb.tile([128, 16, 384], BF16, tag="EGb")
                    self.ETa = expa.tile([128, 2, 512], BF16, tag="ETa")
                    self.ETb = expb.tile([128, 2, 512], BF16, tag="ETb")
                    self.ctx = outp.tile([128, NT, 65], BF16, tag="ctx")

                def ELt(self, t):
                    t %= NT
                    return (self.ELa[:, t, :] if t < 16
                            else self.ELb[:, t - 16, :])

                def EGt(self, t):
                    t %= NT
                    return (self.EGa[:, t, :] if t < 16
                            else self.EGb[:, t - 16, :])

                def ETp(self, p):
                    return (self.ETa[:, p, :] if p < 2
                            else self.ETb[:, p - 2, :])

                def TG(self, g):
                    """transpose group: 8 table-tiles (g=0..3 -> L, 4..7 -> G)
                    via 8 single-tile transposes + 1 copy.  kv K-region units
                    65:161 = (K_hi, K_lo, K_hi) fp8 pairs."""
                    tab, gg = (0, g) if g < 4 else (1, g - 4)
                    tp = auxp.tile([96, 8, 128], BF16, tag="aux")
                    for p in range(8):
                        t = 32 * tab + 8 * gg + p
                        nc.tensor.transpose(
                            out=tp[:, p, :],
                            in_=self.kv[:, t, 65:161],
                            identity=ident[:])
                    kT = self.klT if tab == 0 else self.kgT
                    k3 = kT[:].rearrange("p (t c) -> p t c", c=128)
                    nc.vector.tensor_copy(
                        k3[0:96, 8 * gg:8 * gg + 8, :], tp[:])

                def _exp(self, eng, out_ap, in_ap, ncols):
                    if eng == "S":
                        nc.scalar.activation(out_ap, in_ap, EXP, scale=0.125)
                    elif eng == "D":
                        nc.vector.tensor_tensor(
                            out_ap, base[:].broadcast_to([128, ncols]),
                            in_ap, POW)
                    else:
                        nc.gpsimd.tensor_tensor(
                            out_ap, base[:].broadcast_to([128, ncols]),
                            in_ap, POW)

                def Lp(self, p):
                    """local scores pack: tiles 4p..4p+3, 256-q windows."""
                    st = psL.tile([128, 4, 256], F32, tag="pL")
                    for j in range(4):
                        c = 4 * p + j
                        lhsT = (self.klT[:, 128 * c:128 * c + 128]
                                .bitcast(FP8)
                                .rearrange("p (k two) -> p two k", two=2))
                        rhs = self.q34[:, :, 128 * c + 64:128 * c + 320]
                        nc.tensor.matmul(st[:, j, :], lhsT, rhs,
                                         start=True, stop=True, perf_mode=DR)
                    dst = (self.ELa[:, 4 * p:4 * p + 4, :] if p < 4 else
                           self.ELb[:, 4 * p - 16:4 * p - 12, :])
                    self._exp(L_ENG[p], dst, st[:], 1024)

                def Gp(self, p):
                    """global scores pack: tiles 2p, 2p+1, 384-q windows."""
                    st = psG.tile([128, 2, 512], F32, tag="pG")
                    for j in range(2):
                        t = 2 * p + j
                        lhsT = (self.kgT[:, 128 * t:128 * t + 128]
                                .bitcast(FP8)
                                .rearrange("p (k two) -> p two k", two=2))
                        rhs = self.q34[0:97, :, 128 * t:128 * t + 384]
                        nc.tensor.matmul(st[:, j, 0:384], lhsT, rhs,
                                         start=True, stop=True, perf_mode=DR)
                    dst = (self.EGa[:, 2 * p:2 * p + 2, :] if p < 8 else
                           self.EGb[:, 2 * p - 16:2 * p - 14, :])
                    self._exp(G_ENG[p], dst, st[:, :, 0:384], 768)

                def Tp(self, p):
                    """gtok scores: segments 2p, 2p+1 at partition halves."""
                    st = psT.tile([128, 512], F32, tag="pT")
                    for j in range(2):
                        s = 2 * p + j
                        rhs = self.q34[0:97, :, 128 + 512 * s:128 + 512 * s + 512]
                        nc.tensor.matmul(st[64 * j:64 * j + 64, :],
                                         self.gkTp[:], rhs, start=True,
                                         stop=True, perf_mode=DR,
                                         tile_position=(0, 64 * j))
                    self._exp(T_ENG[p], self.ETp(p), st[:], 512)

                def PVG(self, s):
                    """PV group: q-tiles 4s..4s+3 -> psum [128, 4, 65]."""
                    acc = psPV.tile([128, 512], F32, tag="pv")
                    for jj in range(4):
                        j = 4 * s + jj
                        out = acc[:, 65 * jj:65 * jj + 65]
                        # gtok initializes (full-128 contraction, parity-zero)
                        nc.tensor.matmul(
                            out, self.ETp(j // 8)[:,
                                 128 * (j % 4):128 * (j % 4) + 128],
                            self.gv1[:, (j // 4) % 2, :],
                            start=True, stop=False, skip_group_check=True)
                        for dt_ in (-1, 0, 1):   # global tiles j-1, j, j+1
                            t = (j + dt_) % NT
                            col = 128 * (1 - dt_)
                            nc.tensor.matmul(
                                out, self.EGt(t)[:, col:col + 128],
                                self.kv[:, 32 + t, 0:65],
                                start=False, stop=False, skip_group_check=True)
                        # local tile j (full 128 q)
                        nc.tensor.matmul(
                            out, self.ELt(j)[:, 64:192], self.kv[:, j, 0:65],
                            start=False, stop=False, skip_group_check=True)
                        # local j-1 edge -> q partitions 0:64
                        nc.tensor.matmul(
                            out[0:64, :], self.ELt(j - 1)[:, 192:256],
                            self.kv[:, (j - 1) % NT, 0:65],
                            start=False, stop=False, skip_group_check=True)
                        # local j+1 edge -> q partitions 64:128
                        nc.tensor.matmul(
                            out[64:128, :], self.ELt(j + 1)[:, 0:64],
                            self.kv[:, (j + 1) % NT, 0:65],
                            start=False, stop=(jj == 3),
                            skip_group_check=True)
                    eng = C_ENG[s]
                    dst = self.ctx[:, 4 * s:4 * s + 4, :]
                    src = acc[:, 0:260].rearrange("p (a b) -> p a b", b=65)
                    if eng == "D":
                        nc.vector.tensor_copy(dst, src)
                    else:
                        nc.gpsimd.tensor_copy(dst, src)

                def out_chunk(self, c):
                    # chunks of 8 q-tiles (2 PV groups)
                    nc.sync.dma_start(out_d[self.i][:, 8 * c:8 * c + 8, :],
                                      self.ctx[:, 8 * c:8 * c + 8, :])

            # PE p-state warm-up: ~3us of junk matmuls so real matmuls run at
            # full clock
            warm = psT.tile([128, 512], F32, tag="pT")
            idxbf = idx_sb[:].bitcast(BF16)
            for w in range(7):
                nc.tensor.matmul(warm[:], idxbf[:, 0:128], idxbf[:, 0:512],
                                 start=True, stop=True)

            def head_start(st):
                """TGs + first score packs for a head."""
                st.TG(0)
                st.Lp(0); st.Lp(1)
                st.TG(1)
                st.Lp(2); st.Lp(3)
                st.Tp(0)
                st.TG(4)
                st.Gp(0); st.Gp(1); st.Gp(2); st.Gp(3)
                st.Tp(1)
                st.TG(5)
                st.Gp(4); st.Gp(5); st.Gp(6); st.Gp(7)

            cur = NHState(0, emit_loads(0))
            head_start(cur)

            for i in range(PER_CORE):
                if i + 1 < PER_CORE:
                    with tc.tile_wait_until(0.012 + 0.0145 * i):
                        loads_next = emit_loads(i + 1)
                else:
                    loads_next = None
                cur.TG(2)
                cur.Lp(4)
                cur.TG(6)
                cur.Gp(8); cur.Gp(9)
                cur.Tp(2)
                cur.PVG(1)
                cur.TG(3)
                cur.Lp(5); cur.Gp(10); cur.Gp(11)
                cur.PVG(2)
                cur.Lp(6)
                cur.TG(7)
                cur.Gp(12); cur.Gp(13)
                cur.Tp(3)
                cur.PVG(3)
                cur.out_chunk(1)
                cur.Lp(7); cur.Gp(14); cur.Gp(15)
                cur.PVG(4)
                cur.PVG(5)
                cur.out_chunk(2)
                nxt = NHState(i + 1, loads_next) if loads_next else None
                if nxt is not None:
                    nxt.TG(0)
                    nxt.Lp(0)
                cur.PVG(6)
                if nxt is not None:
                    nxt.Lp(1)
                    nxt.TG(1)
                cur.PVG(7)
                cur.out_chunk(3)
                if nxt is not None:
                    nxt.Lp(2); nxt.Lp(3)
                    nxt.Tp(0)
                    nxt.TG(4)
                cur.PVG(0)
                cur.out_chunk(0)
                if nxt is not None:
                    nxt.Gp(0); nxt.Gp(1); nxt.Gp(2); nxt.Gp(3)
                    nxt.Tp(1)
                    nxt.TG(5)
                    nxt.Gp(4); nxt.Gp(5); nxt.Gp(6); nxt.Gp(7)
                cur = nxt

    nc.compile()
    return nc


_CACHED = None


def _get_program():
    global _CACHED
    if _CACHED is None:
        _CACHED = build_program()
    return _CACHED


def _prep_core_inputs(q, k, v, gk, gv, lidx, gidx, pairs):
    bf = ml_dtypes.bfloat16
    f8 = ml_dtypes.float8_e4m3
    qTh = np.zeros((PER_CORE, 99, 2, QW), dtype=f8)
    kv = np.zeros((PER_CORE, T, 512), dtype=np.uint8)
    gkTp = np.zeros((PER_CORE, 97, 2, 64), dtype=f8)
    gv1 = np.zeros((PER_CORE, 128, 2, 65), dtype=bf)
    idx = np.empty((128, PER_CORE * 512), dtype=np.int16)

    # query-side mask rows: -240 on quadrant ((col-64)//64)%4 pairing the
    # kmask one-hot rows (0,3,2,1)
    jcol = np.arange(QW)
    quad = ((jcol - 64) // 64) % 4
    qmask = np.zeros((4, QW), np.float32)
    for r, qd in enumerate((0, 3, 2, 1)):
        qmask[r, quad == qd] = -240.0

    for s, (n, h) in enumerate(pairs):
        qt = np.ascontiguousarray(q[n, h].T)            # (64, T)
        qhalo = np.concatenate([qt[:, T - 128:], qt, qt[:, :128]], axis=1)
        q_hi = qhalo.astype(f8)
        q_lo = (qhalo - q_hi.astype(np.float32)).astype(f8)
        bias = np.zeros((2, QW), np.float32)
        bias[0, :] = -20.0      # exp bias: logits shift by -2.5
        qq = np.concatenate([q_hi.astype(np.float32),
                             q_hi.astype(np.float32),
                             q_lo.astype(np.float32),
                             bias, qmask], axis=0)      # (198, QW)
        qTh[s] = qq.reshape(99, 2, QW).astype(f8)
        v1 = np.concatenate([v[n, h], np.ones((T, 1), np.float32)],
                            axis=1).astype(bf)          # (T, 65)
        kv[s, :, 0:130] = v1.view(np.uint8)
        k_hi = k[n, h].astype(f8)
        k_lo = (k[n, h] - k_hi.astype(np.float32)).astype(f8)
        kv[s, :, 130:194] = k_hi.view(np.uint8)
        kv[s, :, 194:258] = k_lo.view(np.uint8)
        kv[s, :, 258:322] = k_hi.view(np.uint8)
        gkT = np.ascontiguousarray(gk[n, h].T)          # (64, 64)
        gk_hi = gkT.astype(f8)
        gk_lo = (gkT - gk_hi.astype(np.float32)).astype(f8)
        gg = np.concatenate([gk_hi.astype(np.float32),
                             gk_lo.astype(np.float32),
                             gk_hi.astype(np.float32),
                             np.ones((1, 64), np.float32),
                             np.zeros((1, 64), np.float32)], axis=0)  # (194, 64)
        gkTp[s] = gg.reshape(97, 2, 64).astype(f8)
        g1 = np.concatenate([gv[n, h], np.ones((64, 1), np.float32)],
                            axis=1).astype(bf)
        gv1[s, 0:64, 0] = g1
        gv1[s, 64:128, 1] = g1
        ix = np.concatenate([lidx[n, h, :, 0], gidx[n, h, :, 0]]).astype(np.int16)
        idx[:, 512 * s:512 * (s + 1)] = np.tile(ix.reshape(512, 16).T, (8, 1))

    m = np.arange(T) % 256
    km = np.stack([(m >= 64) & (m < 128), m < 64,
                   m >= 192, (m >= 128) & (m < 192)]).astype(np.float32)
    kmask_u = np.zeros((3, T, 2), f8)
    kmask_u[0, :, 0] = 1.0      # bias row pair (ones, 0)
    for r in range(4):
        kmask_u[1 + r // 2, :, r % 2] = km[r].astype(f8)
    ident = np.eye(128, dtype=bf)
    base = np.full((128, 1), np.exp(0.125), np.float32)
    return {"qTh": qTh, "kv": kv.view(bf), "kmask": kmask_u.view(bf).reshape(3, T),
            "gkTp": gkTp, "gv1": gv1, "idx": idx, "ident": ident, "base": base}


def kernel(query_layer, key_layer, value_layer, attention_mask, local_idx,
           global_idx, global_key, global_value, global_mask):
    # attention_mask / global_mask are all-zero per the input spec
    q = np.asarray(query_layer, np.float32)
    k = np.asarray(key_layer, np.float32)
    v = np.asarray(value_layer, np.float32)
    gk = np.asarray(global_key, np.float32)
    gv = np.asarray(global_value, np.float32)
    li = np.asarray(local_idx)
    gi = np.asarray(global_idx)

    nc = _get_program()
    in_maps = []
    for m in range(NCORES):
        pairs = [((3 * m + s) // H, (3 * m + s) % H) for s in range(PER_CORE)]
        in_maps.append(_prep_core_inputs(q, k, v, gk, gv, li, gi, pairs))
    res = bass_utils.run_bass_kernel_spmd(nc, in_maps, core_ids=list(range(NCORES)))

    out = np.empty((N, H, T, D), np.float32)
    for m in range(NCORES):
        ctxT = np.asarray(res.results[m]["ctxT"], dtype=np.float32)  # (3,128,32,65)
        for s in range(PER_CORE):
            n, h = (3 * m + s) // H, (3 * m + s) % H
            a = ctxT[s].transpose(1, 0, 2).reshape(T, 65)
            out[n, h] = a[:, :64] / a[:, 64:65]
    return out


# revision 13
# speedup vs baseline: 1.0845x; 1.0845x over previous
"""BlockGlobalAttentionProduct Trainium2 kernel (v2).

Sharding: 24 (n,h) pairs across 8 cores, 3 per core. Per (n,h):
  - ONE dma_gather table per head: 512B rows [V1 bf16 130B | K_hi | K_lo |
    K_hi fp8 64B each | pad] (a 512B descriptor costs the same as 256B),
    8192 slots (local 4096 + global 4096) in 2 calls; V1 (with baked ones
    column) is used directly as the PV rhs
  - K^T built by one PE transpose per tile at bf16 granularity (2 fp8
    d-values ride one 16-bit unit), yielding a [96, 2-plane] layout
    (K_hi, K_lo, K_hi) + 3 bias/mask partitions in one copy
  - scores run as fp8 DoubleRow matmuls (0.5 cyc/col) with hi/lo error
    compensation: q ships as (q_hi, q_hi, q_lo) plane pairs so the matmul
    computes q_hi*K_hi + q_hi*K_lo + q_lo*K_hi ~ exact q.K; the local halo
    mask and a -2.5 exp bias fold in as extra partition pairs
  - exp(0.125 s) computed on THREE engines: ScalarE activation, DVE pow, and
    GPSIMD pow (base = e^0.125 broadcast), outputs fp8 probs
  - PV transposed: out[q, 65] += E[keys, q].T @ V1[keys, 65] per (tile,
    q-tile) incidence; gtok contribution initializes each q-tile with the
    gv1 parity trick; ctx copies (PSUM->SBUF bf16) split across DVE/GPSIMD
  - host does the final divide-by-denominator
"""

import sys

sys.path.insert(0, "/opt/trn_rl_repo")

import numpy as np
import ml_dtypes

import concourse.bacc as bacc
import concourse.mybir as mybir
from concourse import bass, tile, bass_utils, library_config

N, H, T, D = 2, 12, 4096, 64
NH = N * H
NCORES = 8
PER_CORE = NH // NCORES   # 3
NT = 32                   # 128-key tiles per table
QW = 128 + T + 128        # q halo cols [-128, T+128)

BF16 = mybir.dt.bfloat16
F32 = mybir.dt.float32
FP8 = mybir.dt.float8e4
I16 = mybir.dt.int16
EXP = mybir.ActivationFunctionType.Exp
DR = mybir.MatmulPerfMode.DoubleRow
POW = mybir.AluOpType.pow

# engine assignment for exp packs / ctx copies: "S" ScalarE, "D" DVE, "P" Pool
L_ENG = ["S", "D", "P", "P", "S", "D", "P", "P"]            # 8 packs of 1024
G_ENG = ["S", "S", "S", "S", "S", "S", "S", "S",
         "D", "D", "D", "S", "P", "P", "S", "D"]            # 16 packs of 768
T_ENG = ["S", "S", "S", "S"]                                # 4 packs of 512
C_ENG = ["P", "D", "P", "D", "P", "D", "P", "D"]            # 8 ctx copies of 260


def build_program():
    nc = bacc.Bacc("TRN2", target_bir_lowering=False, debug=False,
                   num_devices=NCORES)

    qTh_d = nc.dram_tensor("qTh", [PER_CORE, 99, 2, QW], FP8, kind="ExternalInput")
    kv_d = nc.dram_tensor("kv", [PER_CORE, T, 256], BF16, kind="ExternalInput")
    kmask_d = nc.dram_tensor("kmask", [3, T], BF16, kind="ExternalInput")
    gkTp_d = nc.dram_tensor("gkTp", [PER_CORE, 97, 2, 64], FP8, kind="ExternalInput")
    gv1_d = nc.dram_tensor("gv1", [PER_CORE, 128, 2, 65], BF16, kind="ExternalInput")
    idx_d = nc.dram_tensor("idx", [128, PER_CORE * 512], I16, kind="ExternalInput")
    ident_d = nc.dram_tensor("ident", [128, 128], BF16, kind="ExternalInput")
    base_d = nc.dram_tensor("base", [128, 1], F32, kind="ExternalInput")
    out_d = nc.dram_tensor("ctxT", [PER_CORE, 128, 32, 65], BF16,
                           kind="ExternalOutput")

    with tile.TileContext(nc) as tc:
        with (
            tc.tile_pool(name="const", bufs=1) as constp,
            tc.tile_pool(name="land", bufs=2) as land,
            tc.tile_pool(name="kt", bufs=2) as ktp,
            tc.tile_pool(name="expa", bufs=2) as expa,
            tc.tile_pool(name="expb", bufs=1) as expb,
            tc.tile_pool(name="outp", bufs=2) as outp,
            tc.tile_pool(name="psL", bufs=1, space="PSUM") as psL,
            tc.tile_pool(name="psG", bufs=1, space="PSUM") as psG,
            tc.tile_pool(name="psT", bufs=1, space="PSUM") as psT,
            tc.tile_pool(name="psPV", bufs=2, space="PSUM") as psPV,
            tc.tile_pool(name="aux", bufs=1, space="PSUM") as auxp,
        ):
            ident = constp.tile([128, 128], BF16, tag="ident")
            base = constp.tile([128, 1], F32, tag="base")
            idx_sb = constp.tile([128, PER_CORE * 512], I16, tag="idx")
            nc.sync.dma_start(idx_sb[:, 0:768], idx_d[:, 0:768])
            nc.sync.dma_start(idx_sb[:, 768:], idx_d[:, 768:])
            nc.sync.dma_start(ident[:], ident_d[:])
            nc.sync.dma_start(base[:], base_d[:])
            lib_i = nc.gpsimd.load_library(library_config.mlp)

            first_gather = [None]
            last_gather = [None]

            def emit_loads(i, part=False):
                """DMA loads + 2 gather calls for head i."""
                from concourse.tile_rust import add_dep_helper
                q34 = land.tile([99, 2, QW], FP8, tag="q")
                kv = land.tile([128, 64, 256], BF16, tag="kv")
                gkTp = land.tile([97, 2, 64], FP8, tag="gkTp")
                gv1 = land.tile([128, 2, 65], BF16, tag="gv1")
                d1 = nc.sync.dma_start(q34[:], qTh_d[i])
                d2 = nc.sync.dma_start(gkTp[:], gkTp_d[i])
                d3 = nc.sync.dma_start(gv1[:], gv1_d[i])
                if last_gather[0] is not None:
                    for d in (d1, d2, d3):
                        add_dep_helper(d.ins, last_gather[0].ins,
                                       reason="prev gathers before next loads")
                gs = []
                for t in range(2):   # L then G table
                    g = nc.gpsimd.dma_gather(
                        kv[:, 32 * t:32 * t + 32, :], kv_d[i],
                        idx_sb[:, 512 * i + 256 * t:512 * i + 256 * t + 256],
                        4096, 4096, 256, single_packet=False)
                    gs.append(g)
                if first_gather[0] is None:
                    add_dep_helper(lib_i.ins, gs[0].ins, reason="lib first")
                    first_gather[0] = gs[0]
                last_gather[0] = gs[-1]
                return dict(q34=q34, kv=kv, gkTp=gkTp, gv1=gv1)

            class NHState:
                def __init__(self, i, loads):
                    self.i = i
                    self.q34 = loads["q34"]
                    self.kv = loads["kv"]
                    self.gkTp = loads["gkTp"]
                    self.gv1 = loads["gv1"]
                    # K^T tables, bf16-unit layout [34, T]; fp8 view is
                    # [34, 2, T]-plane DoubleRow weights
                    self.klT = ktp.tile([99, T], BF16, tag="klT")
                    self.kgT = ktp.tile([97, T], BF16, tag="kgT")
                    nc.sync.dma_start(self.klT[96:99, :], kmask_d[:])
                    nc.sync.dma_start(self.kgT[96:97, :], kmask_d[0:1, :])
                    self.ELa = expa.tile([128, 16, 256], BF16, tag="ELa")
                    self.ELb = expb.tile([128, 16, 256], BF16, tag="ELb")
                    self.EGa = expa.tile([128, 16, 384], BF16, tag="EGa")
                    self.EGb = expb.tile([128, 16, 384], BF16, tag="EGb")
                    self.ETa = expa.tile([128, 2, 512], BF16, tag="ETa")
                    self.ETb = expb.tile([128, 2, 512], BF16, tag="ETb")
                    self.ctx = outp.tile([128, NT, 65], BF16, tag="ctx")

                def ELt(self, t):
                    t %= NT
                    return (self.ELa[:, t, :] if t < 16
                            else self.ELb[:, t - 16, :])

                def EGt(self, t):
                    t %= NT
                    return (self.EGa[:, t, :] if t < 16
                            else self.EGb[:, t - 16, :])

                def ETp(self, p):
                    return (self.ETa[:, p, :] if p < 2
                            else self.ETb[:, p - 2, :])

                def TG(self, g):
                    """transpose group: 8 table-tiles (g=0..3 -> L, 4..7 -> G)
                    via 8 single-tile transposes + 1 copy.  kv K-region units
                    65:161 = (K_hi, K_lo, K_hi) fp8 pairs."""
                    tab, gg = (0, g) if g < 4 else (1, g - 4)
                    tp = auxp.tile([96, 8, 128], BF16, tag="aux")
                    for p in range(8):
                        t = 32 * tab + 8 * gg + p
                        nc.tensor.transpose(
                            out=tp[:, p, :],
                            in_=self.kv[:, t, 65:161],
                            identity=ident[:])
                    kT = self.klT if tab == 0 else self.kgT
                    k3 = kT[:].rearrange("p (t c) -> p t c", c=128)
                    nc.vector.tensor_copy(
                        k3[0:96, 8 * gg:8 * gg + 8, :], tp[:])

                def _exp(self, eng, out_ap, in_ap, ncols):
                    if eng == "S":
                        nc.scalar.activation(out_ap, in_ap, EXP, scale=0.125)
                    elif eng == "D":
                        nc.vector.tensor_tensor(
                            out_ap, base[:].broadcast_to([128, ncols]),
                            in_ap, POW)
                    else:
                        nc.gpsimd.tensor_tensor(
                            out_ap, base[:].broadcast_to([128, ncols]),
                            in_ap, POW)

                def Lp(self, p):
                    """local scores pack: tiles 4p..4p+3, 256-q windows."""
                    st = psL.tile([128, 4, 256], F32, tag="pL")
                    for j in range(4):
                        c = 4 * p + j
                        lhsT = (self.klT[:, 128 * c:128 * c + 128]
                                .bitcast(FP8)
                                .rearrange("p (k two) -> p two k", two=2))
                        rhs = self.q34[:, :, 128 * c + 64:128 * c + 320]
                        nc.tensor.matmul(st[:, j, :], lhsT, rhs,
                                         start=True, stop=True, perf_mode=DR)
                    dst = (self.ELa[:, 4 * p:4 * p + 4, :] if p < 4 else
                           self.ELb[:, 4 * p - 16:4 * p - 12, :])
                    self._exp(L_ENG[p], dst, st[:], 1024)

                def Gp(self, p):
                    """global scores pack: tiles 2p, 2p+1, 384-q windows."""
                    st = psG.tile([128, 2, 512], F32, tag="pG")
                    for j in range(2):
                        t = 2 * p + j
                        lhsT = (self.kgT[:, 128 * t:128 * t + 128]
                                .bitcast(FP8)
                                .rearrange("p (k two) -> p two k", two=2))
                        rhs = self.q34[0:97, :, 128 * t:128 * t + 384]
                        nc.tensor.matmul(st[:, j, 0:384], lhsT, rhs,
                                         start=True, stop=True, perf_mode=DR)
                    dst = (self.EGa[:, 2 * p:2 * p + 2, :] if p < 8 else
                           self.EGb[:, 2 * p - 16:2 * p - 14, :])
                    self._exp(G_ENG[p], dst, st[:, :, 0:384], 768)

                def Tp(self, p):
                    """gtok scores: segments 2p, 2p+1 at partition halves."""
                    st = psT.tile([128, 512], F32, tag="pT")
                    for j in range(2):
                        s = 2 * p + j
                        rhs = self.q34[0:97, :, 128 + 512 * s:128 + 512 * s + 512]
                        nc.tensor.matmul(st[64 * j:64 * j + 64, :],
                                         self.gkTp[:], rhs, start=True,
                                         stop=True, perf_mode=DR,
                                         tile_position=(0, 64 * j))
                    self._exp(T_ENG[p], self.ETp(p), st[:], 512)

                def PVG(self, s):
                    """PV group: q-tiles 4s..4s+3 -> psum [128, 4, 65]."""
                    acc = psPV.tile([128, 512], F32, tag="pv")
                    for jj in range(4):
                        j = 4 * s + jj
                        out = acc[:, 65 * jj:65 * jj + 65]
                        # gtok initializes (full-128 contraction, parity-zero)
                        nc.tensor.matmul(
                            out, self.ETp(j // 8)[:,
                                 128 * (j % 4):128 * (j % 4) + 128],
                            self.gv1[:, (j // 4) % 2, :],
                            start=True, stop=False, skip_group_check=True)
                        for dt_ in (-1, 0, 1):   # global tiles j-1, j, j+1
                            t = (j + dt_) % NT
                            col = 128 * (1 - dt_)
                            nc.tensor.matmul(
                                out, self.EGt(t)[:, col:col + 128],
                                self.kv[:, 32 + t, 0:65],
                                start=False, stop=False, skip_group_check=True)
                        # local tile j (full 128 q)
                        nc.tensor.matmul(
                            out, self.ELt(j)[:, 64:192], self.kv[:, j, 0:65],
                            start=False, stop=False, skip_group_check=True)
                        # local j-1 edge -> q partitions 0:64
                        nc.tensor.matmul(
                            out[0:64, :], self.ELt(j - 1)[:, 192:256],
                            self.kv[:, (j - 1) % NT, 0:65],
                            start=False, stop=False, skip_group_check=True)
                        # local j+1 edge -> q partitions 64:128
                        nc.tensor.matmul(
                            out[64:128, :], self.ELt(j + 1)[:, 0:64],
                            self.kv[:, (j + 1) % NT, 0:65],
                            start=False, stop=(jj == 3),
                            skip_group_check=True)
                    eng = C_ENG[s]
                    dst = self.ctx[:, 4 * s:4 * s + 4, :]
                    src = acc[:, 0:260].rearrange("p (a b) -> p a b", b=65)
                    if eng == "D":
                        nc.vector.tensor_copy(dst, src)
                    else:
                        nc.gpsimd.tensor_copy(dst, src)

                def out_chunk(self, c):
                    # chunks of 8 q-tiles (2 PV groups)
                    nc.sync.dma_start(out_d[self.i][:, 8 * c:8 * c + 8, :],
                                      self.ctx[:, 8 * c:8 * c + 8, :])

            # PE p-state warm-up: ~3us of junk matmuls so real matmuls run at
            # full clock
            warm = psT.tile([128, 512], F32, tag="pT")
            idxbf = idx_sb[:].bitcast(BF16)
            for w in range(7):
                nc.tensor.matmul(warm[:], idxbf[:, 0:128], idxbf[:, 0:512],
                                 start=True, stop=True)

            def body(cur, nxt, first=False):
                """steady-state emission for head `cur` (G packs + PVGs),
                interleaved with head `nxt`'s L-side start."""
                cur.TG(4); cur.Gp(0); cur.Gp(1)
                cur.TG(5); cur.Gp(2); cur.Gp(3)
                if not first:
                    cur.Tp(0)
                cur.TG(6); cur.Gp(4); cur.Gp(5)
                if not first:
                    cur.Tp(1)
                cur.TG(7); cur.Gp(6); cur.Gp(7)
                cur.PVG(1)
                cur.Gp(8); cur.Gp(9)
                cur.Tp(2)
                cur.PVG(2)
                cur.Gp(10); cur.Gp(11)
                cur.PVG(3); cur.out_chunk(1)
                cur.Gp(12); cur.Gp(13)
                cur.Tp(3)
                cur.PVG(4)
                cur.Gp(14); cur.Gp(15)
                cur.PVG(5); cur.out_chunk(2)
                if nxt is not None:
                    nxt.TG(0); nxt.Lp(0); nxt.Lp(1)
                cur.PVG(6)
                if nxt is not None:
                    nxt.TG(1); nxt.Lp(2); nxt.Lp(3)
                cur.PVG(7); cur.out_chunk(3)
                if nxt is not None:
                    nxt.TG(2); nxt.Lp(4); nxt.Lp(5)
                cur.PVG(0); cur.out_chunk(0)
                if nxt is not None:
                    nxt.TG(3); nxt.Lp(6); nxt.Lp(7)

            # ---- fill: head 0's L-side + gtok before its first body ----
            cur = NHState(0, emit_loads(0))
            warm = psT.tile([128, 512], F32, tag="pT")
            idxbf = idx_sb[:].bitcast(BF16)
            for w in range(7):
                nc.tensor.matmul(warm[:], idxbf[:, 0:128], idxbf[:, 0:512],
                                 start=True, stop=True)
            cur.Tp(0); cur.Tp(1)
            cur.TG(0); cur.Lp(0); cur.Lp(1)
            cur.TG(1); cur.Lp(2); cur.Lp(3)
            cur.TG(2); cur.Lp(4); cur.Lp(5)
            cur.TG(3); cur.Lp(6); cur.Lp(7)

            for i in range(PER_CORE):
                loads_next = emit_loads(i + 1) if i + 1 < PER_CORE else None
                nxt = NHState(i + 1, loads_next) if loads_next else None
                body(cur, nxt, first=(i == 0))
                cur = nxt

    nc.compile()
    return nc


_CACHED = None


def _get_program():
    global _CACHED
    if _CACHED is None:
        _CACHED = build_program()
    return _CACHED


def _prep_core_inputs(q, k, v, gk, gv, lidx, gidx, pairs):
    bf = ml_dtypes.bfloat16
    f8 = ml_dtypes.float8_e4m3
    qTh = np.zeros((PER_CORE, 99, 2, QW), dtype=f8)
    kv = np.zeros((PER_CORE, T, 512), dtype=np.uint8)
    gkTp = np.zeros((PER_CORE, 97, 2, 64), dtype=f8)
    gv1 = np.zeros((PER_CORE, 128, 2, 65), dtype=bf)
    idx = np.empty((128, PER_CORE * 512), dtype=np.int16)

    # query-side mask rows: -240 on quadrant ((col-64)//64)%4 pairing the
    # kmask one-hot rows (0,3,2,1)
    jcol = np.arange(QW)
    quad = ((jcol - 64) // 64) % 4
    qmask = np.zeros((4, QW), np.float32)
    for r, qd in enumerate((0, 3, 2, 1)):
        qmask[r, quad == qd] = -240.0

    for s, (n, h) in enumerate(pairs):
        qt = np.ascontiguousarray(q[n, h].T)            # (64, T)
        qhalo = np.concatenate([qt[:, T - 128:], qt, qt[:, :128]], axis=1)
        q_hi = qhalo.astype(f8)
        q_lo = (qhalo - q_hi.astype(np.float32)).astype(f8)
        bias = np.zeros((2, QW), np.float32)
        bias[0, :] = -20.0      # exp bias: logits shift by -2.5
        qq = np.concatenate([q_hi.astype(np.float32),
                             q_hi.astype(np.float32),
                             q_lo.astype(np.float32),
                             bias, qmask], axis=0)      # (198, QW)
        qTh[s] = qq.reshape(99, 2, QW).astype(f8)
        v1 = np.concatenate([v[n, h], np.ones((T, 1), np.float32)],
                            axis=1).astype(bf)          # (T, 65)
        kv[s, :, 0:130] = v1.view(np.uint8)
        k_hi = k[n, h].astype(f8)
        k_lo = (k[n, h] - k_hi.astype(np.float32)).astype(f8)
        kv[s, :, 130:194] = k_hi.view(np.uint8)
        kv[s, :, 194:258] = k_lo.view(np.uint8)
        kv[s, :, 258:322] = k_hi.view(np.uint8)
        gkT = np.ascontiguousarray(gk[n, h].T)          # (64, 64)
        gk_hi = gkT.astype(f8)
        gk_lo = (gkT - gk_hi.astype(np.float32)).astype(f8)
        gg = np.concatenate([gk_hi.astype(np.float32),
                             gk_lo.astype(np.float32),
                             gk_hi.astype(np.float32),
                             np.ones((1, 64), np.float32),
                             np.zeros((1, 64), np.float32)], axis=0)  # (194, 64)
        gkTp[s] = gg.reshape(97, 2, 64).astype(f8)
        g1 = np.concatenate([gv[n, h], np.ones((64, 1), np.float32)],
                            axis=1).astype(bf)
        gv1[s, 0:64, 0] = g1
        gv1[s, 64:128, 1] = g1
        ix = np.concatenate([lidx[n, h, :, 0], gidx[n, h, :, 0]]).astype(np.int16)
        idx[:, 512 * s:512 * (s + 1)] = np.tile(ix.reshape(512, 16).T, (8, 1))

    m = np.arange(T) % 256
    km = np.stack([(m >= 64) & (m < 128), m < 64,
                   m >= 192, (m >= 128) & (m < 192)]).astype(np.float32)
    kmask_u = np.zeros((3, T, 2), f8)
    kmask_u[0, :, 0] = 1.0      # bias row pair (ones, 0)
    for r in range(4):
        kmask_u[1 + r // 2, :, r % 2] = km[r].astype(f8)
    ident = np.eye(128, dtype=bf)
    base = np.full((128, 1), np.exp(0.125), np.float32)
    return {"qTh": qTh, "kv": kv.view(bf), "kmask": kmask_u.view(bf).reshape(3, T),
            "gkTp": gkTp, "gv1": gv1, "idx": idx, "ident": ident, "base": base}


def kernel(query_layer, key_layer, value_layer, attention_mask, local_idx,
           global_idx, global_key, global_value, global_mask):
    # attention_mask / global_mask are all-zero per the input spec
    q = np.asarray(query_layer, np.float32)
    k = np.asarray(key_layer, np.float32)
    v = np.asarray(value_layer, np.float32)
    gk = np.asarray(global_key, np.float32)
    gv = np.asarray(global_value, np.float32)
    li = np.asarray(local_idx)
    gi = np.asarray(global_idx)

    nc = _get_program()
    in_maps = []
    for m in range(NCORES):
        pairs = [((3 * m + s) // H, (3 * m + s) % H) for s in range(PER_CORE)]
        in_maps.append(_prep_core_inputs(q, k, v, gk, gv, li, gi, pairs))
    res = bass_utils.run_bass_kernel_spmd(nc, in_maps, core_ids=list(range(NCORES)))

    out = np.empty((N, H, T, D), np.float32)
    for m in range(NCORES):
        ctxT = np.asarray(res.results[m]["ctxT"], dtype=np.float32)  # (3,128,32,65)
        for s in range(PER_CORE):
            n, h = (3 * m + s) // H, (3 * m + s) % H
            a = ctxT[s].transpose(1, 0, 2).reshape(T, 65)
            out[n, h] = a[:, :64] / a[:, 64:65]
    return out


# revision 16
# speedup vs baseline: 1.2033x; 1.1095x over previous
"""BlockGlobalAttentionProduct Trainium2 kernel (v2).

Sharding: 24 (n,h) pairs across 8 cores, 3 per core. Per (n,h):
  - ONE dma_gather table per head: 512B rows [V1 bf16 130B | K_hi | K_lo |
    K_hi fp8 64B each | pad] (a 512B descriptor costs the same as 256B),
    8192 slots (local 4096 + global 4096) in 2 calls; V1 (with baked ones
    column) is used directly as the PV rhs
  - K^T built by one PE transpose per tile at bf16 granularity (2 fp8
    d-values ride one 16-bit unit), yielding a [96, 2-plane] layout
    (K_hi, K_lo, K_hi) + 3 bias/mask partitions in one copy
  - scores run as fp8 DoubleRow matmuls (0.5 cyc/col) with hi/lo error
    compensation: q ships as (q_hi, q_hi, q_lo) plane pairs so the matmul
    computes q_hi*K_hi + q_hi*K_lo + q_lo*K_hi ~ exact q.K; the local halo
    mask and a -2.5 exp bias fold in as extra partition pairs
  - exp(0.125 s) computed on THREE engines: ScalarE activation, DVE pow, and
    GPSIMD pow (base = e^0.125 broadcast), outputs fp8 probs
  - PV transposed: out[q, 65] += E[keys, q].T @ V1[keys, 65] per (tile,
    q-tile) incidence; gtok contribution initializes each q-tile with the
    gv1 parity trick; ctx copies (PSUM->SBUF bf16) split across DVE/GPSIMD
  - host does the final divide-by-denominator
"""

import sys

sys.path.insert(0, "/opt/trn_rl_repo")

import numpy as np
import ml_dtypes

import concourse.bacc as bacc
import concourse.mybir as mybir
from concourse import bass, tile, bass_utils, library_config

N, H, T, D = 2, 12, 4096, 64
NH = N * H
NCORES = 8
PER_CORE = NH // NCORES   # 3
NT = 32                   # 128-key tiles per table
QW = 128 + T + 128        # q halo cols [-128, T+128)

BF16 = mybir.dt.bfloat16
F32 = mybir.dt.float32
FP8 = mybir.dt.float8e4
I16 = mybir.dt.int16
EXP = mybir.ActivationFunctionType.Exp
DR = mybir.MatmulPerfMode.DoubleRow
POW = mybir.AluOpType.pow

# engine assignment for exp packs / ctx copies: "S" ScalarE, "D" DVE, "P" Pool
L_ENG = ["S", "D", "P", "S", "D", "P", "S", "D",
         "P", "S", "D", "P", "S", "D", "P", "S"]            # 16 packs of 512
G_ENG = ["S", "S", "D", "S", "S", "D", "S", "P",
         "S", "D", "S", "P", "S", "D", "S", "P"]            # 16 packs of 768
T_ENG = ["S", "S", "S", "S"]                                # 4 packs of 512
C_ENG = ["D", "P", "D", "P", "D", "P", "D", "P",
         "D", "P", "D", "P", "D", "P", "D", "P"]            # 16 ctx copies of 130


def build_program():
    nc = bacc.Bacc("TRN2", target_bir_lowering=False, debug=False,
                   num_devices=NCORES)

    qTh_d = nc.dram_tensor("qTh", [PER_CORE, 99, 2, QW], FP8, kind="ExternalInput")
    kv_d = nc.dram_tensor("kv", [PER_CORE, T, 256], BF16, kind="ExternalInput")
    kmask_d = nc.dram_tensor("kmask", [3, T], BF16, kind="ExternalInput")
    gkTp_d = nc.dram_tensor("gkTp", [PER_CORE, 97, 2, 64], FP8, kind="ExternalInput")
    gv1_d = nc.dram_tensor("gv1", [PER_CORE, 128, 2, 65], BF16, kind="ExternalInput")
    idx_d = nc.dram_tensor("idx", [128, PER_CORE * 512], I16, kind="ExternalInput")
    ident_d = nc.dram_tensor("ident", [128, 128], BF16, kind="ExternalInput")
    base_d = nc.dram_tensor("base", [128, 1], F32, kind="ExternalInput")
    out_d = nc.dram_tensor("ctxT", [PER_CORE, 128, 32, 65], BF16,
                           kind="ExternalOutput")

    with tile.TileContext(nc) as tc:
        with (
            tc.tile_pool(name="const", bufs=1) as constp,
            tc.tile_pool(name="land", bufs=2) as land,
            tc.tile_pool(name="kt", bufs=2) as ktp,
            tc.tile_pool(name="expa", bufs=2) as expa,
            tc.tile_pool(name="expb", bufs=1) as expb,
            tc.tile_pool(name="outp", bufs=2) as outp,
            tc.tile_pool(name="psL", bufs=1, space="PSUM") as psL,
            tc.tile_pool(name="psG", bufs=2, space="PSUM") as psG,
            tc.tile_pool(name="psPV", bufs=2, space="PSUM") as psPV,
            tc.tile_pool(name="aux", bufs=1, space="PSUM") as auxp,
        ):
            ident = constp.tile([128, 128], BF16, tag="ident")
            base = constp.tile([128, 1], F32, tag="base")
            idx_sb = constp.tile([128, PER_CORE * 512], I16, tag="idx")
            nc.sync.dma_start(idx_sb[:, 0:768], idx_d[:, 0:768])
            nc.sync.dma_start(idx_sb[:, 768:], idx_d[:, 768:])
            nc.sync.dma_start(ident[:], ident_d[:])
            nc.sync.dma_start(base[:], base_d[:])
            lib_i = nc.gpsimd.load_library(library_config.mlp)

            first_gather = [None]
            last_gather = [None]

            def emit_loads(i):
                """DMA loads + 2 gather calls for head i."""
                from concourse.tile_rust import add_dep_helper
                q34 = land.tile([99, 2, QW], FP8, tag="q")
                kv = land.tile([128, 64, 256], BF16, tag="kv")
                gkTp = land.tile([97, 2, 64], FP8, tag="gkTp")
                gv1 = land.tile([128, 2, 65], BF16, tag="gv1")
                d1 = nc.sync.dma_start(q34[:], qTh_d[i])
                d2 = nc.sync.dma_start(gkTp[:], gkTp_d[i])
                d3 = nc.sync.dma_start(gv1[:], gv1_d[i])
                if last_gather[0] is not None:
                    for d in (d1, d2, d3):
                        add_dep_helper(d.ins, last_gather[0].ins,
                                       reason="prev gathers before next loads")
                gs = []
                for t in range(2):   # L then G table
                    g = nc.gpsimd.dma_gather(
                        kv[:, 32 * t:32 * t + 32, :], kv_d[i],
                        idx_sb[:, 512 * i + 256 * t:512 * i + 256 * t + 256],
                        4096, 4096, 256, single_packet=False)
                    gs.append(g)
                if first_gather[0] is None:
                    add_dep_helper(lib_i.ins, gs[0].ins, reason="lib first")
                    first_gather[0] = gs[0]
                last_gather[0] = gs[-1]
                return dict(q34=q34, kv=kv, gkTp=gkTp, gv1=gv1)

            class NHState:
                def __init__(self, i, loads):
                    self.i = i
                    self.q34 = loads["q34"]
                    self.kv = loads["kv"]
                    self.gkTp = loads["gkTp"]
                    self.gv1 = loads["gv1"]
                    # K^T tables, bf16-unit layout: fp8 view is the
                    # (K_hi, K_lo, K_hi) x 2-plane DoubleRow weight layout
                    self.klT = ktp.tile([99, T], BF16, tag="klT")
                    self.kgT = ktp.tile([97, T], BF16, tag="kgT")
                    nc.sync.dma_start(self.klT[96:99, :], kmask_d[:])
                    nc.sync.dma_start(self.kgT[96:97, :], kmask_d[0:1, :])
                    self.ELa = expa.tile([128, 16, 256], BF16, tag="ELa")
                    self.ELb = expb.tile([128, 16, 256], BF16, tag="ELb")
                    self.EGa = expa.tile([128, 16, 384], BF16, tag="EGa")
                    self.EGb = expb.tile([128, 16, 384], BF16, tag="EGb")
                    self.ETa = expa.tile([128, 2, 512], BF16, tag="ETa")
                    self.ETb = expb.tile([128, 2, 512], BF16, tag="ETb")
                    self.ctx = outp.tile([128, NT, 65], BF16, tag="ctx")

                def ELt(self, t):
                    t %= NT
                    return (self.ELa[:, t, :] if t < 16
                            else self.ELb[:, t - 16, :])

                def EGt(self, t):
                    t %= NT
                    return (self.EGa[:, t, :] if t < 16
                            else self.EGb[:, t - 16, :])

                def ETp(self, p):
                    return (self.ETa[:, p, :] if p < 2
                            else self.ETb[:, p - 2, :])

                def TG(self, g):
                    """transpose group: 8 table-tiles (g=0..3 -> L, 4..7 -> G)
                    via 8 single-tile transposes + 1 copy.  kv K-region units
                    65:161 = (K_hi, K_lo, K_hi) fp8 pairs."""
                    tab, gg = (0, g) if g < 4 else (1, g - 4)
                    tp = auxp.tile([96, 8, 128], BF16, tag="aux")
                    for p in range(8):
                        t = 32 * tab + 8 * gg + p
                        nc.tensor.transpose(
                            out=tp[:, p, :],
                            in_=self.kv[:, t, 65:161],
                            identity=ident[:])
                    kT = self.klT if tab == 0 else self.kgT
                    k3 = kT[:].rearrange("p (t c) -> p t c", c=128)
                    nc.vector.tensor_copy(
                        k3[0:96, 8 * gg:8 * gg + 8, :], tp[:])

                def _exp(self, eng, out_ap, in_ap, ncols):
                    if eng == "S":
                        nc.scalar.activation(out_ap, in_ap, EXP, scale=0.125)
                    elif eng == "D":
                        nc.vector.tensor_tensor(
                            out_ap, base[:].broadcast_to([128, ncols]),
                            in_ap, POW)
                    else:
                        nc.gpsimd.tensor_tensor(
                            out_ap, base[:].broadcast_to([128, ncols]),
                            in_ap, POW)

                def Lp(self, p):
                    """local scores pack: tiles 2p, 2p+1, 256-q windows."""
                    st = psL.tile([128, 2, 256], F32, tag="pL")
                    for j in range(2):
                        c = 2 * p + j
                        lhsT = (self.klT[:, 128 * c:128 * c + 128]
                                .bitcast(FP8)
                                .rearrange("p (k two) -> p two k", two=2))
                        rhs = self.q34[:, :, 128 * c + 64:128 * c + 320]
                        nc.tensor.matmul(st[:, j, :], lhsT, rhs,
                                         start=True, stop=True, perf_mode=DR)
                    dst = (self.ELa[:, 2 * p:2 * p + 2, :] if p < 8 else
                           self.ELb[:, 2 * p - 16:2 * p - 14, :])
                    self._exp(L_ENG[p], dst, st[:], 512)

                def Gp(self, p):
                    """global scores pack: tiles 2p, 2p+1, 384-q windows."""
                    st = psG.tile([128, 2, 512], F32, tag="pG")
                    for j in range(2):
                        t = 2 * p + j
                        lhsT = (self.kgT[:, 128 * t:128 * t + 128]
                                .bitcast(FP8)
                                .rearrange("p (k two) -> p two k", two=2))
                        rhs = self.q34[0:97, :, 128 * t:128 * t + 384]
                        nc.tensor.matmul(st[:, j, 0:384], lhsT, rhs,
                                         start=True, stop=True, perf_mode=DR)
                    dst = (self.EGa[:, 2 * p:2 * p + 2, :] if p < 8 else
                           self.EGb[:, 2 * p - 16:2 * p - 14, :])
                    self._exp(G_ENG[p], dst, st[:, :, 0:384], 768)

                def Tp(self, p):
                    """gtok scores: segments 2p, 2p+1 at partition halves."""
                    st = psG.tile([128, 512], F32, tag="pG")
                    for j in range(2):
                        s = 2 * p + j
                        rhs = self.q34[0:97, :, 128 + 512 * s:128 + 512 * s + 512]
                        nc.tensor.matmul(st[64 * j:64 * j + 64, :],
                                         self.gkTp[:], rhs, start=True,
                                         stop=True, perf_mode=DR,
                                         tile_position=(0, 64 * j))
                    self._exp(T_ENG[p], self.ETp(p), st[:], 512)

                def PVG(self, u):
                    """PV group: q-tiles 2u, 2u+1 -> psum [128, 2, 65]."""
                    acc = psPV.tile([128, 2, 65], F32, tag="pv")
                    for jj in range(2):
                        j = 2 * u + jj
                        out = acc[:, jj, :]
                        # gtok initializes (full-128 contraction, parity-zero)
                        nc.tensor.matmul(
                            out, self.ETp(j // 8)[:,
                                 128 * (j % 4):128 * (j % 4) + 128],
                            self.gv1[:, (j // 4) % 2, :],
                            start=True, stop=False, skip_group_check=True)
                        for dt_ in (-1, 0, 1):   # global tiles j-1, j, j+1
                            t = (j + dt_) % NT
                            col = 128 * (1 - dt_)
                            nc.tensor.matmul(
                                out, self.EGt(t)[:, col:col + 128],
                                self.kv[:, 32 + t, 0:65],
                                start=False, stop=False, skip_group_check=True)
                        # local tile j (full 128 q)
                        nc.tensor.matmul(
                            out, self.ELt(j)[:, 64:192], self.kv[:, j, 0:65],
                            start=False, stop=False, skip_group_check=True)
                        # local j-1 edge -> q partitions 0:64
                        nc.tensor.matmul(
                            out[0:64, :], self.ELt(j - 1)[:, 192:256],
                            self.kv[:, (j - 1) % NT, 0:65],
                            start=False, stop=False, skip_group_check=True)
                        # local j+1 edge -> q partitions 64:128
                        nc.tensor.matmul(
                            out[64:128, :], self.ELt(j + 1)[:, 0:64],
                            self.kv[:, (j + 1) % NT, 0:65],
                            start=False, stop=(jj == 1),
                            skip_group_check=True)
                    eng = C_ENG[u]
                    dst = self.ctx[:, 2 * u:2 * u + 2, :]
                    if eng == "D":
                        nc.vector.tensor_copy(dst, acc[:])
                    else:
                        nc.gpsimd.tensor_copy(dst, acc[:])

                def out_chunk(self, c):
                    # chunks of 8 q-tiles (4 PV groups)
                    nc.sync.dma_start(out_d[self.i][:, 8 * c:8 * c + 8, :],
                                      self.ctx[:, 8 * c:8 * c + 8, :])

            def body(cur, nxt):
                """Steady-state emission for head `cur`: G/T scores + PVGs,
                interleaved with head `nxt`'s L-side + gtok start."""
                cur.TG(4); cur.Gp(0)
                cur.TG(5); cur.Gp(1)
                cur.TG(6); cur.Gp(2)
                cur.TG(7); cur.Gp(3)
                cur.Tp(2); cur.Lp(11); cur.Gp(4)
                cur.Tp(3); cur.Lp(12); cur.Gp(5)
                cur.PVG(1); cur.Lp(13); cur.Gp(6)
                cur.PVG(2); cur.Lp(14); cur.Gp(7)
                cur.PVG(3); cur.Lp(15); cur.Gp(8)
                cur.PVG(4); cur.Gp(9)
                cur.PVG(5); cur.Gp(10)
                cur.PVG(6); cur.Gp(11)
                cur.PVG(7); cur.out_chunk(1); cur.Gp(12)
                cur.PVG(8); cur.Gp(13)
                cur.PVG(9); cur.Gp(14)
                cur.PVG(10); cur.Gp(15)
                cur.PVG(11); cur.out_chunk(2)
                if nxt is not None:
                    nxt.TG(0); nxt.Lp(0)
                cur.PVG(12)
                if nxt is not None:
                    nxt.Lp(1); nxt.TG(1); nxt.Lp(2)
                cur.PVG(13)
                if nxt is not None:
                    nxt.Lp(3); nxt.TG(2); nxt.Lp(4)
                cur.PVG(14)
                if nxt is not None:
                    nxt.Lp(5); nxt.TG(3); nxt.Lp(6)
                cur.PVG(15); cur.out_chunk(3)
                if nxt is not None:
                    nxt.Lp(7); nxt.Tp(0); nxt.Lp(8)
                cur.PVG(0); cur.out_chunk(0)
                if nxt is not None:
                    nxt.Lp(9); nxt.Tp(1); nxt.Lp(10)

            # ---- fill: head 0's L-side + gtok before its first body ----
            cur = NHState(0, emit_loads(0))
            warm = psG.tile([128, 512], F32, tag="pG")
            idxbf = idx_sb[:].bitcast(BF16)
            for w in range(7):
                nc.tensor.matmul(warm[:], idxbf[:, 0:128], idxbf[:, 0:512],
                                 start=True, stop=True)
            cur.Tp(0); cur.Tp(1)
            cur.TG(0); cur.Lp(0); cur.Lp(1)
            cur.TG(1); cur.Lp(2); cur.Lp(3)
            cur.TG(2); cur.Lp(4); cur.Lp(5)
            cur.TG(3); cur.Lp(6); cur.Lp(7)
            cur.Lp(8); cur.Lp(9); cur.Lp(10); cur.Lp(11)

            for i in range(PER_CORE):
                loads_next = emit_loads(i + 1) if i + 1 < PER_CORE else None
                nxt = NHState(i + 1, loads_next) if loads_next else None
                body(cur, nxt)
                cur = nxt

    nc.compile()
    return nc


_CACHED = None


def _get_program():
    global _CACHED
    if _CACHED is None:
        _CACHED = build_program()
    return _CACHED


def _prep_core_inputs(q, k, v, gk, gv, lidx, gidx, pairs):
    bf = ml_dtypes.bfloat16
    f8 = ml_dtypes.float8_e4m3
    qTh = np.zeros((PER_CORE, 99, 2, QW), dtype=f8)
    kv = np.zeros((PER_CORE, T, 512), dtype=np.uint8)
    gkTp = np.zeros((PER_CORE, 97, 2, 64), dtype=f8)
    gv1 = np.zeros((PER_CORE, 128, 2, 65), dtype=bf)
    idx = np.empty((128, PER_CORE * 512), dtype=np.int16)

    # query-side mask rows: -240 on quadrant ((col-64)//64)%4 pairing the
    # kmask one-hot rows (0,3,2,1)
    jcol = np.arange(QW)
    quad = ((jcol - 64) // 64) % 4
    qmask = np.zeros((4, QW), np.float32)
    for r, qd in enumerate((0, 3, 2, 1)):
        qmask[r, quad == qd] = -240.0

    for s, (n, h) in enumerate(pairs):
        qt = np.ascontiguousarray(q[n, h].T)            # (64, T)
        qhalo = np.concatenate([qt[:, T - 128:], qt, qt[:, :128]], axis=1)
        q_hi = qhalo.astype(f8)
        q_lo = (qhalo - q_hi.astype(np.float32)).astype(f8)
        bias = np.zeros((2, QW), np.float32)
        bias[0, :] = -20.0      # exp bias: logits shift by -2.5
        qq = np.concatenate([q_hi.astype(np.float32),
                             q_hi.astype(np.float32),
                             q_lo.astype(np.float32),
                             bias, qmask], axis=0)      # (198, QW)
        qTh[s] = qq.reshape(99, 2, QW).astype(f8)
        v1 = np.concatenate([v[n, h], np.ones((T, 1), np.float32)],
                            axis=1).astype(bf)          # (T, 65)
        kv[s, :, 0:130] = v1.view(np.uint8)
        k_hi = k[n, h].astype(f8)
        k_lo = (k[n, h] - k_hi.astype(np.float32)).astype(f8)
        kv[s, :, 130:194] = k_hi.view(np.uint8)
        kv[s, :, 194:258] = k_lo.view(np.uint8)
        kv[s, :, 258:322] = k_hi.view(np.uint8)
        gkT = np.ascontiguousarray(gk[n, h].T)          # (64, 64)
        gk_hi = gkT.astype(f8)
        gk_lo = (gkT - gk_hi.astype(np.float32)).astype(f8)
        gg = np.concatenate([gk_hi.astype(np.float32),
                             gk_lo.astype(np.float32),
                             gk_hi.astype(np.float32),
                             np.ones((1, 64), np.float32),
                             np.zeros((1, 64), np.float32)], axis=0)  # (194, 64)
        gkTp[s] = gg.reshape(97, 2, 64).astype(f8)
        g1 = np.concatenate([gv[n, h], np.ones((64, 1), np.float32)],
                            axis=1).astype(bf)
        gv1[s, 0:64, 0] = g1
        gv1[s, 64:128, 1] = g1
        ix = np.concatenate([lidx[n, h, :, 0], gidx[n, h, :, 0]]).astype(np.int16)
        idx[:, 512 * s:512 * (s + 1)] = np.tile(ix.reshape(512, 16).T, (8, 1))

    m = np.arange(T) % 256
    km = np.stack([(m >= 64) & (m < 128), m < 64,
                   m >= 192, (m >= 128) & (m < 192)]).astype(np.float32)
    kmask_u = np.zeros((3, T, 2), f8)
    kmask_u[0, :, 0] = 1.0      # bias row pair (ones, 0)
    for r in range(4):
        kmask_u[1 + r // 2, :, r % 2] = km[r].astype(f8)
    ident = np.eye(128, dtype=bf)
    base = np.full((128, 1), np.exp(0.125), np.float32)
    return {"qTh": qTh, "kv": kv.view(bf), "kmask": kmask_u.view(bf).reshape(3, T),
            "gkTp": gkTp, "gv1": gv1, "idx": idx, "ident": ident, "base": base}


def kernel(query_layer, key_layer, value_layer, attention_mask, local_idx,
           global_idx, global_key, global_value, global_mask):
    # attention_mask / global_mask are all-zero per the input spec
    q = np.asarray(query_layer, np.float32)
    k = np.asarray(key_layer, np.float32)
    v = np.asarray(value_layer, np.float32)
    gk = np.asarray(global_key, np.float32)
    gv = np.asarray(global_value, np.float32)
    li = np.asarray(local_idx)
    gi = np.asarray(global_idx)

    nc = _get_program()
    in_maps = []
    for m in range(NCORES):
        pairs = [((3 * m + s) // H, (3 * m + s) % H) for s in range(PER_CORE)]
        in_maps.append(_prep_core_inputs(q, k, v, gk, gv, li, gi, pairs))
    res = bass_utils.run_bass_kernel_spmd(nc, in_maps, core_ids=list(range(NCORES)))

    out = np.empty((N, H, T, D), np.float32)
    for m in range(NCORES):
        ctxT = np.asarray(res.results[m]["ctxT"], dtype=np.float32)  # (3,128,32,65)
        for s in range(PER_CORE):
            n, h = (3 * m + s) // H, (3 * m + s) % H
            a = ctxT[s].transpose(1, 0, 2).reshape(T, 65)
            out[n, h] = a[:, :64] / a[:, 64:65]
    return out


# revision 17
# speedup vs baseline: 1.2339x; 1.0255x over previous
"""BlockGlobalAttentionProduct Trainium2 kernel (v2).

Sharding: 24 (n,h) pairs across 8 cores, 3 per core. Per (n,h):
  - ONE dma_gather table per head: 512B rows [V1 bf16 130B | K_hi | K_lo |
    K_hi fp8 64B each | pad] (a 512B descriptor costs the same as 256B),
    8192 slots (local 4096 + global 4096) in 2 calls; V1 (with baked ones
    column) is used directly as the PV rhs
  - K^T built by one PE transpose per tile at bf16 granularity (2 fp8
    d-values ride one 16-bit unit), yielding a [96, 2-plane] layout
    (K_hi, K_lo, K_hi) + 3 bias/mask partitions in one copy
  - scores run as fp8 DoubleRow matmuls (0.5 cyc/col) with hi/lo error
    compensation: q ships as (q_hi, q_hi, q_lo) plane pairs so the matmul
    computes q_hi*K_hi + q_hi*K_lo + q_lo*K_hi ~ exact q.K; the local halo
    mask and a -2.5 exp bias fold in as extra partition pairs
  - exp(0.125 s) computed on THREE engines: ScalarE activation, DVE pow, and
    GPSIMD pow (base = e^0.125 broadcast), outputs fp8 probs
  - PV transposed: out[q, 65] += E[keys, q].T @ V1[keys, 65] per (tile,
    q-tile) incidence; gtok contribution initializes each q-tile with the
    gv1 parity trick; ctx copies (PSUM->SBUF bf16) split across DVE/GPSIMD
  - host does the final divide-by-denominator
"""

import sys

sys.path.insert(0, "/opt/trn_rl_repo")

import numpy as np
import ml_dtypes

import concourse.bacc as bacc
import concourse.mybir as mybir
from concourse import bass, tile, bass_utils, library_config

N, H, T, D = 2, 12, 4096, 64
NH = N * H
NCORES = 8
PER_CORE = NH // NCORES   # 3
NT = 32                   # 128-key tiles per table
QW = 128 + T + 128        # q halo cols [-128, T+128)

BF16 = mybir.dt.bfloat16
F32 = mybir.dt.float32
FP8 = mybir.dt.float8e4
I16 = mybir.dt.int16
EXP = mybir.ActivationFunctionType.Exp
DR = mybir.MatmulPerfMode.DoubleRow
POW = mybir.AluOpType.pow

# engine assignment for exp packs / ctx copies: "S" ScalarE, "D" DVE, "P" Pool
L_ENG = ["S", "D", "P", "S", "D", "P", "S", "D",
         "P", "S", "D", "P", "S", "D", "P", "S"]            # 16 packs of 512
G_ENG = ["S", "S", "D", "S", "S", "D", "S", "P",
         "S", "D", "S", "P", "S", "D", "S", "P"]            # 16 packs of 768
T_ENG = ["S", "S", "S", "S"]                                # 4 packs of 512
C_ENG = ["D", "P", "D", "P", "D", "P", "D", "P",
         "D", "P", "D", "P", "D", "P", "D", "P"]            # 16 ctx copies of 130


def build_program():
    nc = bacc.Bacc("TRN2", target_bir_lowering=False, debug=False,
                   num_devices=NCORES)

    qTh_d = nc.dram_tensor("qTh", [PER_CORE, 99, 2, QW], FP8, kind="ExternalInput")
    kv_d = nc.dram_tensor("kv", [PER_CORE, T, 256], BF16, kind="ExternalInput")
    kmask_d = nc.dram_tensor("kmask", [3, T], BF16, kind="ExternalInput")
    gkTp_d = nc.dram_tensor("gkTp", [PER_CORE, 97, 2, 64], FP8, kind="ExternalInput")
    gv1_d = nc.dram_tensor("gv1", [PER_CORE, 128, 2, 65], BF16, kind="ExternalInput")
    idx_d = nc.dram_tensor("idx", [128, PER_CORE * 512], I16, kind="ExternalInput")
    ident_d = nc.dram_tensor("ident", [128, 128], BF16, kind="ExternalInput")
    base_d = nc.dram_tensor("base", [128, 1], F32, kind="ExternalInput")
    out_d = nc.dram_tensor("ctxT", [PER_CORE, 128, 32, 65], BF16,
                           kind="ExternalOutput")

    with tile.TileContext(nc) as tc:
        with (
            tc.tile_pool(name="const", bufs=1) as constp,
            tc.tile_pool(name="land", bufs=2) as land,
            tc.tile_pool(name="kt", bufs=2) as ktp,
            tc.tile_pool(name="expa", bufs=2) as expa,
            tc.tile_pool(name="expb", bufs=1) as expb,
            tc.tile_pool(name="outp", bufs=2) as outp,
            tc.tile_pool(name="psL", bufs=1, space="PSUM") as psL,
            tc.tile_pool(name="psG", bufs=2, space="PSUM") as psG,
            tc.tile_pool(name="psPV", bufs=2, space="PSUM") as psPV,
            tc.tile_pool(name="aux", bufs=1, space="PSUM") as auxp,
        ):
            ident = constp.tile([128, 128], BF16, tag="ident")
            base = constp.tile([128, 1], F32, tag="base")
            idx_sb = constp.tile([128, PER_CORE * 512], I16, tag="idx")
            nc.sync.dma_start(idx_sb[:, 0:768], idx_d[:, 0:768])
            nc.sync.dma_start(idx_sb[:, 768:], idx_d[:, 768:])
            lib_i = nc.gpsimd.load_library(library_config.mlp)

            first_gather = [None]
            last_gather = [None]

            def emit_loads(i):
                """DMA loads + 2 gather calls for head i."""
                from concourse.tile_rust import add_dep_helper
                q34 = land.tile([99, 2, QW], FP8, tag="q")
                kv = land.tile([128, 64, 256], BF16, tag="kv")
                gkTp = land.tile([97, 2, 64], FP8, tag="gkTp")
                gv1 = land.tile([128, 2, 65], BF16, tag="gv1")
                d1 = nc.sync.dma_start(q34[:], qTh_d[i])
                d2 = nc.sync.dma_start(gkTp[:], gkTp_d[i])
                d3 = nc.sync.dma_start(gv1[:], gv1_d[i])
                if last_gather[0] is not None:
                    for d in (d1, d2, d3):
                        add_dep_helper(d.ins, last_gather[0].ins,
                                       reason="prev gathers before next loads")
                gs = []
                for t in range(2):   # L then G table
                    g = nc.gpsimd.dma_gather(
                        kv[:, 32 * t:32 * t + 32, :], kv_d[i],
                        idx_sb[:, 512 * i + 256 * t:512 * i + 256 * t + 256],
                        4096, 4096, 256, single_packet=False)
                    gs.append(g)
                if first_gather[0] is None:
                    add_dep_helper(lib_i.ins, gs[0].ins, reason="lib first")
                    first_gather[0] = gs[0]
                last_gather[0] = gs[-1]
                return dict(q34=q34, kv=kv, gkTp=gkTp, gv1=gv1)

            class NHState:
                def __init__(self, i, loads):
                    self.i = i
                    self.q34 = loads["q34"]
                    self.kv = loads["kv"]
                    self.gkTp = loads["gkTp"]
                    self.gv1 = loads["gv1"]
                    # K^T tables, bf16-unit layout: fp8 view is the
                    # (K_hi, K_lo, K_hi) x 2-plane DoubleRow weight layout
                    self.klT = ktp.tile([99, T], BF16, tag="klT")
                    self.kgT = ktp.tile([97, T], BF16, tag="kgT")
                    nc.sync.dma_start(self.klT[96:99, :], kmask_d[:])
                    nc.sync.dma_start(self.kgT[96:97, :], kmask_d[0:1, :])
                    self.ELa = expa.tile([128, 16, 256], BF16, tag="ELa")
                    self.ELb = expb.tile([128, 16, 256], BF16, tag="ELb")
                    self.EGa = expa.tile([128, 16, 384], BF16, tag="EGa")
                    self.EGb = expb.tile([128, 16, 384], BF16, tag="EGb")
                    self.ETa = expa.tile([128, 2, 512], BF16, tag="ETa")
                    self.ETb = expb.tile([128, 2, 512], BF16, tag="ETb")
                    self.ctx = outp.tile([128, NT, 65], BF16, tag="ctx")

                def ELt(self, t):
                    t %= NT
                    return (self.ELa[:, t, :] if t < 16
                            else self.ELb[:, t - 16, :])

                def EGt(self, t):
                    t %= NT
                    return (self.EGa[:, t, :] if t < 16
                            else self.EGb[:, t - 16, :])

                def ETp(self, p):
                    return (self.ETa[:, p, :] if p < 2
                            else self.ETb[:, p - 2, :])

                def TG(self, g):
                    """transpose group: 8 table-tiles (g=0..3 -> L, 4..7 -> G)
                    via 8 single-tile transposes + 1 copy.  kv K-region units
                    65:161 = (K_hi, K_lo, K_hi) fp8 pairs."""
                    tab, gg = (0, g) if g < 4 else (1, g - 4)
                    tp = auxp.tile([96, 8, 128], BF16, tag="aux")
                    for p in range(8):
                        t = 32 * tab + 8 * gg + p
                        nc.tensor.transpose(
                            out=tp[:, p, :],
                            in_=self.kv[:, t, 65:161],
                            identity=ident[:])
                    kT = self.klT if tab == 0 else self.kgT
                    k3 = kT[:].rearrange("p (t c) -> p t c", c=128)
                    nc.vector.tensor_copy(
                        k3[0:96, 8 * gg:8 * gg + 8, :], tp[:])

                def _exp(self, eng, out_ap, in_ap, ncols):
                    if eng == "S":
                        nc.scalar.activation(out_ap, in_ap, EXP, scale=0.125)
                    elif eng == "D":
                        nc.vector.tensor_tensor(
                            out_ap, base[:].broadcast_to([128, ncols]),
                            in_ap, POW)
                    else:
                        nc.gpsimd.tensor_tensor(
                            out_ap, base[:].broadcast_to([128, ncols]),
                            in_ap, POW)

                def Lp(self, p):
                    """local scores pack: tiles 2p, 2p+1, 256-q windows."""
                    st = psL.tile([128, 2, 256], F32, tag="pL")
                    for j in range(2):
                        c = 2 * p + j
                        lhsT = (self.klT[:, 128 * c:128 * c + 128]
                                .bitcast(FP8)
                                .rearrange("p (k two) -> p two k", two=2))
                        rhs = self.q34[:, :, 128 * c + 64:128 * c + 320]
                        nc.tensor.matmul(st[:, j, :], lhsT, rhs,
                                         start=True, stop=True, perf_mode=DR)
                    dst = (self.ELa[:, 2 * p:2 * p + 2, :] if p < 8 else
                           self.ELb[:, 2 * p - 16:2 * p - 14, :])
                    self._exp(L_ENG[p], dst, st[:], 512)

                def Gp(self, p):
                    """global scores pack: tiles 2p, 2p+1, 384-q windows."""
                    st = psG.tile([128, 2, 512], F32, tag="pG")
                    for j in range(2):
                        t = 2 * p + j
                        lhsT = (self.kgT[:, 128 * t:128 * t + 128]
                                .bitcast(FP8)
                                .rearrange("p (k two) -> p two k", two=2))
                        rhs = self.q34[0:97, :, 128 * t:128 * t + 384]
                        nc.tensor.matmul(st[:, j, 0:384], lhsT, rhs,
                                         start=True, stop=True, perf_mode=DR)
                    dst = (self.EGa[:, 2 * p:2 * p + 2, :] if p < 8 else
                           self.EGb[:, 2 * p - 16:2 * p - 14, :])
                    self._exp(G_ENG[p], dst, st[:, :, 0:384], 768)

                def Tp(self, p):
                    """gtok scores: segments 2p, 2p+1 at partition halves."""
                    st = psG.tile([128, 512], F32, tag="pG")
                    for j in range(2):
                        s = 2 * p + j
                        rhs = self.q34[0:97, :, 128 + 512 * s:128 + 512 * s + 512]
                        nc.tensor.matmul(st[64 * j:64 * j + 64, :],
                                         self.gkTp[:], rhs, start=True,
                                         stop=True, perf_mode=DR,
                                         tile_position=(0, 64 * j))
                    self._exp(T_ENG[p], self.ETp(p), st[:], 512)

                def PVG(self, u):
                    """PV group: q-tiles 2u, 2u+1 -> psum [128, 2, 65]."""
                    acc = psPV.tile([128, 2, 65], F32, tag="pv")
                    for jj in range(2):
                        j = 2 * u + jj
                        out = acc[:, jj, :]
                        # gtok initializes (full-128 contraction, parity-zero)
                        nc.tensor.matmul(
                            out, self.ETp(j // 8)[:,
                                 128 * (j % 4):128 * (j % 4) + 128],
                            self.gv1[:, (j // 4) % 2, :],
                            start=True, stop=False, skip_group_check=True)
                        for dt_ in (-1, 0, 1):   # global tiles j-1, j, j+1
                            t = (j + dt_) % NT
                            col = 128 * (1 - dt_)
                            nc.tensor.matmul(
                                out, self.EGt(t)[:, col:col + 128],
                                self.kv[:, 32 + t, 0:65],
                                start=False, stop=False, skip_group_check=True)
                        # local tile j (full 128 q)
                        nc.tensor.matmul(
                            out, self.ELt(j)[:, 64:192], self.kv[:, j, 0:65],
                            start=False, stop=False, skip_group_check=True)
                        # local j-1 edge -> q partitions 0:64
                        nc.tensor.matmul(
                            out[0:64, :], self.ELt(j - 1)[:, 192:256],
                            self.kv[:, (j - 1) % NT, 0:65],
                            start=False, stop=False, skip_group_check=True)
                        # local j+1 edge -> q partitions 64:128
                        nc.tensor.matmul(
                            out[64:128, :], self.ELt(j + 1)[:, 0:64],
                            self.kv[:, (j + 1) % NT, 0:65],
                            start=False, stop=(jj == 1),
                            skip_group_check=True)
                    eng = C_ENG[u]
                    dst = self.ctx[:, 2 * u:2 * u + 2, :]
                    if eng == "D":
                        nc.vector.tensor_copy(dst, acc[:])
                    else:
                        nc.gpsimd.tensor_copy(dst, acc[:])

                def out_chunk(self, c):
                    # chunks of 8 q-tiles (4 PV groups)
                    nc.sync.dma_start(out_d[self.i][:, 8 * c:8 * c + 8, :],
                                      self.ctx[:, 8 * c:8 * c + 8, :])

            def body(cur, nxt, prev):
                """Steady-state emission for head `cur`: G/T scores + PVGs,
                interleaved with head `nxt`'s L-side + gtok start; `prev`'s
                output DMAs go out first (they are emitted here, after the
                next loads, so they never block the load DMAs in the queue)."""
                if prev is not None:
                    for c in range(4):
                        prev.out_chunk(c)
                cur.TG(4); cur.Gp(0)
                cur.TG(5); cur.Gp(1)
                cur.TG(6); cur.Gp(2)
                cur.TG(7); cur.Gp(3)
                cur.Tp(2); cur.Lp(11); cur.Gp(4)
                cur.Tp(3); cur.Lp(12); cur.Gp(5)
                cur.PVG(1); cur.Lp(13); cur.Gp(6)
                cur.PVG(2); cur.Lp(14); cur.Gp(7)
                cur.PVG(3); cur.Lp(15); cur.Gp(8)
                cur.PVG(4); cur.Gp(9)
                cur.PVG(5); cur.Gp(10)
                cur.PVG(6); cur.Gp(11)
                cur.PVG(7); cur.Gp(12)
                cur.PVG(8); cur.Gp(13)
                cur.PVG(9); cur.Gp(14)
                cur.PVG(10); cur.Gp(15)
                cur.PVG(11)
                if nxt is not None:
                    nxt.TG(0); nxt.Lp(0)
                cur.PVG(12)
                if nxt is not None:
                    nxt.Lp(1); nxt.TG(1); nxt.Lp(2)
                cur.PVG(13)
                if nxt is not None:
                    nxt.Lp(3); nxt.TG(2); nxt.Lp(4)
                cur.PVG(14)
                if nxt is not None:
                    nxt.Lp(5); nxt.TG(3); nxt.Lp(6)
                cur.PVG(15)
                if nxt is not None:
                    nxt.Lp(7); nxt.Tp(0); nxt.Lp(8)
                cur.PVG(0)
                if nxt is not None:
                    nxt.Lp(9); nxt.Tp(1); nxt.Lp(10)

            # ---- fill: head 0's L-side + gtok before its first body ----
            cur = NHState(0, emit_loads(0))
            nc.sync.dma_start(ident[:], ident_d[:])
            nc.sync.dma_start(base[:], base_d[:])
            warm = psG.tile([128, 512], F32, tag="pG")
            idxbf = idx_sb[:].bitcast(BF16)
            for w in range(7):
                nc.tensor.matmul(warm[:], idxbf[:, 0:128], idxbf[:, 0:512],
                                 start=True, stop=True)
            cur.Tp(0); cur.Tp(1)
            cur.TG(0); cur.Lp(0); cur.Lp(1)
            cur.TG(1); cur.Lp(2); cur.Lp(3)
            cur.TG(2); cur.Lp(4); cur.Lp(5)
            cur.TG(3); cur.Lp(6); cur.Lp(7)
            cur.Lp(8); cur.Lp(9); cur.Lp(10); cur.Lp(11)

            prev = None
            for i in range(PER_CORE):
                loads_next = emit_loads(i + 1) if i + 1 < PER_CORE else None
                nxt = NHState(i + 1, loads_next) if loads_next else None
                body(cur, nxt, prev)
                prev = cur
                cur = nxt
            for c in range(4):
                prev.out_chunk(c)

    nc.compile()
    return nc


_CACHED = None


def _get_program():
    global _CACHED
    if _CACHED is None:
        _CACHED = build_program()
    return _CACHED


def _prep_core_inputs(q, k, v, gk, gv, lidx, gidx, pairs):
    bf = ml_dtypes.bfloat16
    f8 = ml_dtypes.float8_e4m3
    qTh = np.zeros((PER_CORE, 99, 2, QW), dtype=f8)
    kv = np.zeros((PER_CORE, T, 512), dtype=np.uint8)
    gkTp = np.zeros((PER_CORE, 97, 2, 64), dtype=f8)
    gv1 = np.zeros((PER_CORE, 128, 2, 65), dtype=bf)
    idx = np.empty((128, PER_CORE * 512), dtype=np.int16)

    # query-side mask rows: -240 on quadrant ((col-64)//64)%4 pairing the
    # kmask one-hot rows (0,3,2,1)
    jcol = np.arange(QW)
    quad = ((jcol - 64) // 64) % 4
    qmask = np.zeros((4, QW), np.float32)
    for r, qd in enumerate((0, 3, 2, 1)):
        qmask[r, quad == qd] = -240.0

    for s, (n, h) in enumerate(pairs):
        qt = np.ascontiguousarray(q[n, h].T)            # (64, T)
        qhalo = np.concatenate([qt[:, T - 128:], qt, qt[:, :128]], axis=1)
        q_hi = qhalo.astype(f8)
        q_lo = (qhalo - q_hi.astype(np.float32)).astype(f8)
        bias = np.zeros((2, QW), np.float32)
        bias[0, :] = -20.0      # exp bias: logits shift by -2.5
        qq = np.concatenate([q_hi.astype(np.float32),
                             q_hi.astype(np.float32),
                             q_lo.astype(np.float32),
                             bias, qmask], axis=0)      # (198, QW)
        qTh[s] = qq.reshape(99, 2, QW).astype(f8)
        v1 = np.concatenate([v[n, h], np.ones((T, 1), np.float32)],
                            axis=1).astype(bf)          # (T, 65)
        kv[s, :, 0:130] = v1.view(np.uint8)
        k_hi = k[n, h].astype(f8)
        k_lo = (k[n, h] - k_hi.astype(np.float32)).astype(f8)
        kv[s, :, 130:194] = k_hi.view(np.uint8)
        kv[s, :, 194:258] = k_lo.view(np.uint8)
        kv[s, :, 258:322] = k_hi.view(np.uint8)
        gkT = np.ascontiguousarray(gk[n, h].T)          # (64, 64)
        gk_hi = gkT.astype(f8)
        gk_lo = (gkT - gk_hi.astype(np.float32)).astype(f8)
        gg = np.concatenate([gk_hi.astype(np.float32),
                             gk_lo.astype(np.float32),
                             gk_hi.astype(np.float32),
                             np.ones((1, 64), np.float32),
                             np.zeros((1, 64), np.float32)], axis=0)  # (194, 64)
        gkTp[s] = gg.reshape(97, 2, 64).astype(f8)
        g1 = np.concatenate([gv[n, h], np.ones((64, 1), np.float32)],
                            axis=1).astype(bf)
        gv1[s, 0:64, 0] = g1
        gv1[s, 64:128, 1] = g1
        ix = np.concatenate([lidx[n, h, :, 0], gidx[n, h, :, 0]]).astype(np.int16)
        idx[:, 512 * s:512 * (s + 1)] = np.tile(ix.reshape(512, 16).T, (8, 1))

    m = np.arange(T) % 256
    km = np.stack([(m >= 64) & (m < 128), m < 64,
                   m >= 192, (m >= 128) & (m < 192)]).astype(np.float32)
    kmask_u = np.zeros((3, T, 2), f8)
    kmask_u[0, :, 0] = 1.0      # bias row pair (ones, 0)
    for r in range(4):
        kmask_u[1 + r // 2, :, r % 2] = km[r].astype(f8)
    ident = np.eye(128, dtype=bf)
    base = np.full((128, 1), np.exp(0.125), np.float32)
    return {"qTh": qTh, "kv": kv.view(bf), "kmask": kmask_u.view(bf).reshape(3, T),
            "gkTp": gkTp, "gv1": gv1, "idx": idx, "ident": ident, "base": base}


def kernel(query_layer, key_layer, value_layer, attention_mask, local_idx,
           global_idx, global_key, global_value, global_mask):
    # attention_mask / global_mask are all-zero per the input spec
    q = np.asarray(query_layer, np.float32)
    k = np.asarray(key_layer, np.float32)
    v = np.asarray(value_layer, np.float32)
    gk = np.asarray(global_key, np.float32)
    gv = np.asarray(global_value, np.float32)
    li = np.asarray(local_idx)
    gi = np.asarray(global_idx)

    nc = _get_program()
    in_maps = []
    for m in range(NCORES):
        pairs = [((3 * m + s) // H, (3 * m + s) % H) for s in range(PER_CORE)]
        in_maps.append(_prep_core_inputs(q, k, v, gk, gv, li, gi, pairs))
    res = bass_utils.run_bass_kernel_spmd(nc, in_maps, core_ids=list(range(NCORES)))

    out = np.empty((N, H, T, D), np.float32)
    for m in range(NCORES):
        ctxT = np.asarray(res.results[m]["ctxT"], dtype=np.float32)  # (3,128,32,65)
        for s in range(PER_CORE):
            n, h = (3 * m + s) // H, (3 * m + s) % H
            a = ctxT[s].transpose(1, 0, 2).reshape(T, 65)
            out[n, h] = a[:, :64] / a[:, 64:65]
    return out


# revision 18
# speedup vs baseline: 1.2363x; 1.0019x over previous
"""BlockGlobalAttentionProduct Trainium2 kernel (v2).

Sharding: 24 (n,h) pairs across 8 cores, 3 per core. Per (n,h):
  - ONE dma_gather table per head: 512B rows [V1 bf16 130B | K_hi | K_lo |
    K_hi fp8 64B each | pad] (a 512B descriptor costs the same as 256B),
    8192 slots (local 4096 + global 4096) in 2 calls; V1 (with baked ones
    column) is used directly as the PV rhs
  - K^T built by one PE transpose per tile at bf16 granularity (2 fp8
    d-values ride one 16-bit unit), yielding a [96, 2-plane] layout
    (K_hi, K_lo, K_hi) + 3 bias/mask partitions in one copy
  - scores run as fp8 DoubleRow matmuls (0.5 cyc/col) with hi/lo error
    compensation: q ships as (q_hi, q_hi, q_lo) plane pairs so the matmul
    computes q_hi*K_hi + q_hi*K_lo + q_lo*K_hi ~ exact q.K; the local halo
    mask and a -2.5 exp bias fold in as extra partition pairs
  - exp(0.125 s) computed on THREE engines: ScalarE activation, DVE pow, and
    GPSIMD pow (base = e^0.125 broadcast), outputs fp8 probs
  - PV transposed: out[q, 65] += E[keys, q].T @ V1[keys, 65] per (tile,
    q-tile) incidence; gtok contribution initializes each q-tile with the
    gv1 parity trick; ctx copies (PSUM->SBUF bf16) split across DVE/GPSIMD
  - host does the final divide-by-denominator
"""

import sys

sys.path.insert(0, "/opt/trn_rl_repo")

import numpy as np
import ml_dtypes

import concourse.bacc as bacc
import concourse.mybir as mybir
from concourse import bass, tile, bass_utils, library_config

N, H, T, D = 2, 12, 4096, 64
NH = N * H
NCORES = 8
PER_CORE = NH // NCORES   # 3
NT = 32                   # 128-key tiles per table
QW = 128 + T + 128        # q halo cols [-128, T+128)

BF16 = mybir.dt.bfloat16
F32 = mybir.dt.float32
FP8 = mybir.dt.float8e4
I16 = mybir.dt.int16
EXP = mybir.ActivationFunctionType.Exp
DR = mybir.MatmulPerfMode.DoubleRow
POW = mybir.AluOpType.pow

# engine assignment for exp packs / ctx copies: "S" ScalarE, "D" DVE, "P" Pool
L_ENG = ["S", "D", "P", "S", "D", "P", "S", "D",
         "P", "S", "D", "P", "S", "D", "P", "S"]            # 16 packs of 512
G_ENG = ["S", "S", "D", "S", "S", "D", "S", "P",
         "S", "D", "S", "P", "S", "D", "S", "P"]            # 16 packs of 768
T_ENG = ["S", "S", "S", "S"]                                # 4 packs of 512
C_ENG = ["D", "P", "D", "P", "D", "P", "D", "P",
         "D", "P", "D", "P", "D", "P", "D", "P"]            # 16 ctx copies of 130


def build_program():
    nc = bacc.Bacc("TRN2", target_bir_lowering=False, debug=False,
                   num_devices=NCORES)

    qTh_d = nc.dram_tensor("qTh", [PER_CORE, 99, 2, QW], FP8, kind="ExternalInput")
    kv_d = nc.dram_tensor("kv", [PER_CORE, T, 256], BF16, kind="ExternalInput")
    kmask_d = nc.dram_tensor("kmask", [3, T], BF16, kind="ExternalInput")
    gkTp_d = nc.dram_tensor("gkTp", [PER_CORE, 97, 2, 64], FP8, kind="ExternalInput")
    gv1_d = nc.dram_tensor("gv1", [PER_CORE, 128, 2, 65], BF16, kind="ExternalInput")
    idx_d = nc.dram_tensor("idx", [128, PER_CORE * 512], I16, kind="ExternalInput")
    ident_d = nc.dram_tensor("ident", [128, 128], BF16, kind="ExternalInput")
    base_d = nc.dram_tensor("base", [128, 1], F32, kind="ExternalInput")
    out_d = nc.dram_tensor("ctxT", [PER_CORE, 128, 32, 65], BF16,
                           kind="ExternalOutput")

    with tile.TileContext(nc) as tc:
        with (
            tc.tile_pool(name="const", bufs=1) as constp,
            tc.tile_pool(name="landq", bufs=3) as landq,
            tc.tile_pool(name="landkv", bufs=2) as landkv,
            tc.tile_pool(name="kt", bufs=2) as ktp,
            tc.tile_pool(name="expa", bufs=2) as expa,
            tc.tile_pool(name="expb", bufs=1) as expb,
            tc.tile_pool(name="outp", bufs=2) as outp,
            tc.tile_pool(name="psL", bufs=1, space="PSUM") as psL,
            tc.tile_pool(name="psG", bufs=2, space="PSUM") as psG,
            tc.tile_pool(name="psPV", bufs=2, space="PSUM") as psPV,
            tc.tile_pool(name="aux", bufs=1, space="PSUM") as auxp,
        ):
            ident = constp.tile([128, 128], BF16, tag="ident")
            base = constp.tile([128, 1], F32, tag="base")
            idx_sb = constp.tile([128, PER_CORE * 512], I16, tag="idx")
            nc.sync.dma_start(idx_sb[:, 0:768], idx_d[:, 0:768])
            nc.sync.dma_start(idx_sb[:, 768:], idx_d[:, 768:])
            lib_i = nc.gpsimd.load_library(library_config.mlp)

            first_gather = [None]
            last_gather = [None]

            def emit_loads(i):
                """DMA loads + 2 gather calls for head i."""
                from concourse.tile_rust import add_dep_helper
                q34 = landq.tile([99, 2, QW], FP8, tag="q")
                kv = landkv.tile([128, 64, 256], BF16, tag="kv")
                gkTp = landq.tile([97, 2, 64], FP8, tag="gkTp")
                gv1 = landq.tile([128, 2, 65], BF16, tag="gv1")
                d1 = nc.sync.dma_start(q34[:], qTh_d[i])
                d2 = nc.sync.dma_start(gkTp[:], gkTp_d[i])
                d3 = nc.sync.dma_start(gv1[:], gv1_d[i])
                if last_gather[0] is not None:
                    for d in (d1, d2, d3):
                        add_dep_helper(d.ins, last_gather[0].ins,
                                       reason="prev gathers before next loads")
                gs = []
                for t in range(2):   # L then G table
                    g = nc.gpsimd.dma_gather(
                        kv[:, 32 * t:32 * t + 32, :], kv_d[i],
                        idx_sb[:, 512 * i + 256 * t:512 * i + 256 * t + 256],
                        4096, 4096, 256, single_packet=False)
                    gs.append(g)
                if first_gather[0] is None:
                    add_dep_helper(lib_i.ins, gs[0].ins, reason="lib first")
                    first_gather[0] = gs[0]
                last_gather[0] = gs[-1]
                return dict(q34=q34, kv=kv, gkTp=gkTp, gv1=gv1)

            class NHState:
                def __init__(self, i, loads):
                    self.i = i
                    self.q34 = loads["q34"]
                    self.kv = loads["kv"]
                    self.gkTp = loads["gkTp"]
                    self.gv1 = loads["gv1"]
                    # K^T tables, bf16-unit layout: fp8 view is the
                    # (K_hi, K_lo, K_hi) x 2-plane DoubleRow weight layout
                    self.klT = ktp.tile([99, T], BF16, tag="klT")
                    self.kgT = ktp.tile([97, T], BF16, tag="kgT")
                    nc.sync.dma_start(self.klT[96:99, :], kmask_d[:])
                    nc.sync.dma_start(self.kgT[96:97, :], kmask_d[0:1, :])
                    self.ELa = expa.tile([128, 16, 256], BF16, tag="ELa")
                    self.ELb = expb.tile([128, 16, 256], BF16, tag="ELb")
                    self.EGa = expa.tile([128, 16, 384], BF16, tag="EGa")
                    self.EGb = expb.tile([128, 16, 384], BF16, tag="EGb")
                    self.ETa = expa.tile([128, 2, 512], BF16, tag="ETa")
                    self.ETb = expb.tile([128, 2, 512], BF16, tag="ETb")
                    self.ctx = outp.tile([128, NT, 65], BF16, tag="ctx")

                def ELt(self, t):
                    t %= NT
                    return (self.ELa[:, t, :] if t < 16
                            else self.ELb[:, t - 16, :])

                def EGt(self, t):
                    t %= NT
                    return (self.EGa[:, t, :] if t < 16
                            else self.EGb[:, t - 16, :])

                def ETp(self, p):
                    return (self.ETa[:, p, :] if p < 2
                            else self.ETb[:, p - 2, :])

                def TG(self, g):
                    """transpose group: 8 table-tiles (g=0..3 -> L, 4..7 -> G)
                    via 8 single-tile transposes + 1 copy.  kv K-region units
                    65:161 = (K_hi, K_lo, K_hi) fp8 pairs."""
                    tab, gg = (0, g) if g < 4 else (1, g - 4)
                    tp = auxp.tile([96, 8, 128], BF16, tag="aux")
                    for p in range(8):
                        t = 32 * tab + 8 * gg + p
                        nc.tensor.transpose(
                            out=tp[:, p, :],
                            in_=self.kv[:, t, 65:161],
                            identity=ident[:])
                    kT = self.klT if tab == 0 else self.kgT
                    k3 = kT[:].rearrange("p (t c) -> p t c", c=128)
                    nc.vector.tensor_copy(
                        k3[0:96, 8 * gg:8 * gg + 8, :], tp[:])

                def _exp(self, eng, out_ap, in_ap, ncols):
                    if eng == "S":
                        nc.scalar.activation(out_ap, in_ap, EXP, scale=0.125)
                    elif eng == "D":
                        nc.vector.tensor_tensor(
                            out_ap, base[:].broadcast_to([128, ncols]),
                            in_ap, POW)
                    else:
                        nc.gpsimd.tensor_tensor(
                            out_ap, base[:].broadcast_to([128, ncols]),
                            in_ap, POW)

                def Lp(self, p):
                    """local scores pack: tiles 2p, 2p+1, 256-q windows."""
                    st = psL.tile([128, 2, 256], F32, tag="pL")
                    for j in range(2):
                        c = 2 * p + j
                        lhsT = (self.klT[:, 128 * c:128 * c + 128]
                                .bitcast(FP8)
                                .rearrange("p (k two) -> p two k", two=2))
                        rhs = self.q34[:, :, 128 * c + 64:128 * c + 320]
                        nc.tensor.matmul(st[:, j, :], lhsT, rhs,
                                         start=True, stop=True, perf_mode=DR)
                    dst = (self.ELa[:, 2 * p:2 * p + 2, :] if p < 8 else
                           self.ELb[:, 2 * p - 16:2 * p - 14, :])
                    self._exp(L_ENG[p], dst, st[:], 512)

                def Gp(self, p):
                    """global scores pack: tiles 2p, 2p+1, 384-q windows."""
                    st = psG.tile([128, 2, 512], F32, tag="pG")
                    for j in range(2):
                        t = 2 * p + j
                        lhsT = (self.kgT[:, 128 * t:128 * t + 128]
                                .bitcast(FP8)
                                .rearrange("p (k two) -> p two k", two=2))
                        rhs = self.q34[0:97, :, 128 * t:128 * t + 384]
                        nc.tensor.matmul(st[:, j, 0:384], lhsT, rhs,
                                         start=True, stop=True, perf_mode=DR)
                    dst = (self.EGa[:, 2 * p:2 * p + 2, :] if p < 8 else
                           self.EGb[:, 2 * p - 16:2 * p - 14, :])
                    self._exp(G_ENG[p], dst, st[:, :, 0:384], 768)

                def Tp(self, p):
                    """gtok scores: segments 2p, 2p+1 at partition halves."""
                    st = psG.tile([128, 512], F32, tag="pG")
                    for j in range(2):
                        s = 2 * p + j
                        rhs = self.q34[0:97, :, 128 + 512 * s:128 + 512 * s + 512]
                        nc.tensor.matmul(st[64 * j:64 * j + 64, :],
                                         self.gkTp[:], rhs, start=True,
                                         stop=True, perf_mode=DR,
                                         tile_position=(0, 64 * j))
                    self._exp(T_ENG[p], self.ETp(p), st[:], 512)

                def PVG(self, u):
                    """PV group: q-tiles 2u, 2u+1 -> psum [128, 2, 65]."""
                    acc = psPV.tile([128, 2, 65], F32, tag="pv")
                    for jj in range(2):
                        j = 2 * u + jj
                        out = acc[:, jj, :]
                        # gtok initializes (full-128 contraction, parity-zero)
                        nc.tensor.matmul(
                            out, self.ETp(j // 8)[:,
                                 128 * (j % 4):128 * (j % 4) + 128],
                            self.gv1[:, (j // 4) % 2, :],
                            start=True, stop=False, skip_group_check=True)
                        for dt_ in (-1, 0, 1):   # global tiles j-1, j, j+1
                            t = (j + dt_) % NT
                            col = 128 * (1 - dt_)
                            nc.tensor.matmul(
                                out, self.EGt(t)[:, col:col + 128],
                                self.kv[:, 32 + t, 0:65],
                                start=False, stop=False, skip_group_check=True)
                        # local tile j (full 128 q)
                        nc.tensor.matmul(
                            out, self.ELt(j)[:, 64:192], self.kv[:, j, 0:65],
                            start=False, stop=False, skip_group_check=True)
                        # local j-1 edge -> q partitions 0:64
                        nc.tensor.matmul(
                            out[0:64, :], self.ELt(j - 1)[:, 192:256],
                            self.kv[:, (j - 1) % NT, 0:65],
                            start=False, stop=False, skip_group_check=True)
                        # local j+1 edge -> q partitions 64:128
                        nc.tensor.matmul(
                            out[64:128, :], self.ELt(j + 1)[:, 0:64],
                            self.kv[:, (j + 1) % NT, 0:65],
                            start=False, stop=(jj == 1),
                            skip_group_check=True)
                    eng = C_ENG[u]
                    dst = self.ctx[:, 2 * u:2 * u + 2, :]
                    if eng == "D":
                        nc.vector.tensor_copy(dst, acc[:])
                    else:
                        nc.gpsimd.tensor_copy(dst, acc[:])

                def out_chunk(self, c):
                    # chunks of 8 q-tiles (4 PV groups)
                    nc.sync.dma_start(out_d[self.i][:, 8 * c:8 * c + 8, :],
                                      self.ctx[:, 8 * c:8 * c + 8, :])

            def body(cur, nxt, prev):
                """Steady-state emission for head `cur`: G/T scores + PVGs,
                interleaved with head `nxt`'s L-side + gtok start; `prev`'s
                output DMAs go out first (they are emitted here, after the
                next loads, so they never block the load DMAs in the queue)."""
                if prev is not None:
                    for c in range(4):
                        prev.out_chunk(c)
                cur.TG(4); cur.Gp(0)
                cur.TG(5); cur.Gp(1)
                cur.TG(6); cur.Gp(2)
                cur.TG(7); cur.Gp(3)
                cur.Tp(2); cur.Lp(11); cur.Gp(4)
                cur.Tp(3); cur.Lp(12); cur.Gp(5)
                cur.PVG(1); cur.Lp(13); cur.Gp(6)
                cur.PVG(2); cur.Lp(14); cur.Gp(7)
                cur.PVG(3); cur.Lp(15); cur.Gp(8)
                cur.PVG(4); cur.Gp(9)
                cur.PVG(5); cur.Gp(10)
                cur.PVG(6); cur.Gp(11)
                cur.PVG(7); cur.Gp(12)
                cur.PVG(8); cur.Gp(13)
                cur.PVG(9); cur.Gp(14)
                cur.PVG(10); cur.Gp(15)
                cur.PVG(11)
                if nxt is not None:
                    nxt.TG(0); nxt.Lp(0)
                cur.PVG(12)
                if nxt is not None:
                    nxt.Lp(1); nxt.TG(1); nxt.Lp(2)
                cur.PVG(13)
                if nxt is not None:
                    nxt.Lp(3); nxt.TG(2); nxt.Lp(4)
                cur.PVG(14)
                if nxt is not None:
                    nxt.Lp(5); nxt.TG(3); nxt.Lp(6)
                cur.PVG(15)
                if nxt is not None:
                    nxt.Lp(7); nxt.Tp(0); nxt.Lp(8)
                cur.PVG(0)
                if nxt is not None:
                    nxt.Lp(9); nxt.Tp(1); nxt.Lp(10)

            # ---- fill: head 0's L-side + gtok before its first body ----
            cur = NHState(0, emit_loads(0))
            nc.sync.dma_start(ident[:], ident_d[:])
            nc.sync.dma_start(base[:], base_d[:])
            warm = psG.tile([128, 512], F32, tag="pG")
            idxbf = idx_sb[:].bitcast(BF16)
            for w in range(7):
                nc.tensor.matmul(warm[:], idxbf[:, 0:128], idxbf[:, 0:512],
                                 start=True, stop=True)
            cur.Tp(0); cur.Tp(1)
            cur.TG(0); cur.Lp(0); cur.Lp(1)
            cur.TG(1); cur.Lp(2); cur.Lp(3)
            cur.TG(2); cur.Lp(4); cur.Lp(5)
            cur.TG(3); cur.Lp(6); cur.Lp(7)
            cur.Lp(8); cur.Lp(9); cur.Lp(10); cur.Lp(11)

            prev = None
            for i in range(PER_CORE):
                loads_next = emit_loads(i + 1) if i + 1 < PER_CORE else None
                nxt = NHState(i + 1, loads_next) if loads_next else None
                body(cur, nxt, prev)
                prev = cur
                cur = nxt
            for c in range(4):
                prev.out_chunk(c)

    nc.compile()
    return nc


_CACHED = None


def _get_program():
    global _CACHED
    if _CACHED is None:
        _CACHED = build_program()
    return _CACHED


def _prep_core_inputs(q, k, v, gk, gv, lidx, gidx, pairs):
    bf = ml_dtypes.bfloat16
    f8 = ml_dtypes.float8_e4m3
    qTh = np.zeros((PER_CORE, 99, 2, QW), dtype=f8)
    kv = np.zeros((PER_CORE, T, 512), dtype=np.uint8)
    gkTp = np.zeros((PER_CORE, 97, 2, 64), dtype=f8)
    gv1 = np.zeros((PER_CORE, 128, 2, 65), dtype=bf)
    idx = np.empty((128, PER_CORE * 512), dtype=np.int16)

    # query-side mask rows: -240 on quadrant ((col-64)//64)%4 pairing the
    # kmask one-hot rows (0,3,2,1)
    jcol = np.arange(QW)
    quad = ((jcol - 64) // 64) % 4
    qmask = np.zeros((4, QW), np.float32)
    for r, qd in enumerate((0, 3, 2, 1)):
        qmask[r, quad == qd] = -240.0

    for s, (n, h) in enumerate(pairs):
        qt = np.ascontiguousarray(q[n, h].T)            # (64, T)
        qhalo = np.concatenate([qt[:, T - 128:], qt, qt[:, :128]], axis=1)
        q_hi = qhalo.astype(f8)
        q_lo = (qhalo - q_hi.astype(np.float32)).astype(f8)
        bias = np.zeros((2, QW), np.float32)
        bias[0, :] = -20.0      # exp bias: logits shift by -2.5
        qq = np.concatenate([q_hi.astype(np.float32),
                             q_hi.astype(np.float32),
                             q_lo.astype(np.float32),
                             bias, qmask], axis=0)      # (198, QW)
        qTh[s] = qq.reshape(99, 2, QW).astype(f8)
        v1 = np.concatenate([v[n, h], np.ones((T, 1), np.float32)],
                            axis=1).astype(bf)          # (T, 65)
        kv[s, :, 0:130] = v1.view(np.uint8)
        k_hi = k[n, h].astype(f8)
        k_lo = (k[n, h] - k_hi.astype(np.float32)).astype(f8)
        kv[s, :, 130:194] = k_hi.view(np.uint8)
        kv[s, :, 194:258] = k_lo.view(np.uint8)
        kv[s, :, 258:322] = k_hi.view(np.uint8)
        gkT = np.ascontiguousarray(gk[n, h].T)          # (64, 64)
        gk_hi = gkT.astype(f8)
        gk_lo = (gkT - gk_hi.astype(np.float32)).astype(f8)
        gg = np.concatenate([gk_hi.astype(np.float32),
                             gk_lo.astype(np.float32),
                             gk_hi.astype(np.float32),
                             np.ones((1, 64), np.float32),
                             np.zeros((1, 64), np.float32)], axis=0)  # (194, 64)
        gkTp[s] = gg.reshape(97, 2, 64).astype(f8)
        g1 = np.concatenate([gv[n, h], np.ones((64, 1), np.float32)],
                            axis=1).astype(bf)
        gv1[s, 0:64, 0] = g1
        gv1[s, 64:128, 1] = g1
        ix = np.concatenate([lidx[n, h, :, 0], gidx[n, h, :, 0]]).astype(np.int16)
        idx[:, 512 * s:512 * (s + 1)] = np.tile(ix.reshape(512, 16).T, (8, 1))

    m = np.arange(T) % 256
    km = np.stack([(m >= 64) & (m < 128), m < 64,
                   m >= 192, (m >= 128) & (m < 192)]).astype(np.float32)
    kmask_u = np.zeros((3, T, 2), f8)
    kmask_u[0, :, 0] = 1.0      # bias row pair (ones, 0)
    for r in range(4):
        kmask_u[1 + r // 2, :, r % 2] = km[r].astype(f8)
    ident = np.eye(128, dtype=bf)
    base = np.full((128, 1), np.exp(0.125), np.float32)
    return {"qTh": qTh, "kv": kv.view(bf), "kmask": kmask_u.view(bf).reshape(3, T),
            "gkTp": gkTp, "gv1": gv1, "idx": idx, "ident": ident, "base": base}


def kernel(query_layer, key_layer, value_layer, attention_mask, local_idx,
           global_idx, global_key, global_value, global_mask):
    # attention_mask / global_mask are all-zero per the input spec
    q = np.asarray(query_layer, np.float32)
    k = np.asarray(key_layer, np.float32)
    v = np.asarray(value_layer, np.float32)
    gk = np.asarray(global_key, np.float32)
    gv = np.asarray(global_value, np.float32)
    li = np.asarray(local_idx)
    gi = np.asarray(global_idx)

    nc = _get_program()
    in_maps = []
    for m in range(NCORES):
        pairs = [((3 * m + s) // H, (3 * m + s) % H) for s in range(PER_CORE)]
        in_maps.append(_prep_core_inputs(q, k, v, gk, gv, li, gi, pairs))
    res = bass_utils.run_bass_kernel_spmd(nc, in_maps, core_ids=list(range(NCORES)))

    out = np.empty((N, H, T, D), np.float32)
    for m in range(NCORES):
        ctxT = np.asarray(res.results[m]["ctxT"], dtype=np.float32)  # (3,128,32,65)
        for s in range(PER_CORE):
            n, h = (3 * m + s) // H, (3 * m + s) % H
            a = ctxT[s].transpose(1, 0, 2).reshape(T, 65)
            out[n, h] = a[:, :64] / a[:, 64:65]
    return out
